# revision 1
# baseline (speedup 1.0000x reference)
"""Trainium2 Bass kernel for a post-norm transformer encoder layer.

Contract: kernel(**inputs) takes the FULL fp32 inputs (as produced by the
problem's setup_inputs) and returns the FULL [2, 2048, 512] fp32 output.

Sharding (8 cores, no collectives): core c owns 512 query tokens of batch
c // 4 (slice (c % 4) * 512). Each core recomputes the K/V projections for
its whole batch (2048 tokens) and runs attention + FFN for its 512 queries.

On-device layout is feature-major [feature, token]; the host pre-transposes
all activations and weights so the device never transposes anything.
"""

import numpy as np
import ml_dtypes

D = 512
S = 2048
B = 2
H = 8
HD = 64
F = 2048
EPS = 1e-5
NCORES = 8
SQ = 512          # queries per core
P = 128           # partitions
KD = D // P       # 4   D-tiles
KT = S // P       # 16  key tiles
TB = S // 512     # 4   512-token blocks
FT = F // P       # 16  FFN hidden tiles

BF16 = ml_dtypes.bfloat16

_CACHE = {}
LAST_RESULT = None


def _build_nc():
    import concourse.bacc as bacc
    import concourse.tile as tile
    from concourse import mybir

    bf = mybir.dt.bfloat16
    f32 = mybir.dt.float32
    ACT = mybir.ActivationFunctionType

    nc = bacc.Bacc("TRN2", target_bir_lowering=False, debug=False)

    def din(name, shape, dt=bf):
        return nc.dram_tensor(name, shape, dt, kind="ExternalInput").ap()

    t_qbf = din("qt_bf", [P, KD, SQ])
    t_qf32 = din("qt_f32", [P, KD, SQ], f32)
    t_kt = din("kt", [P, KD, S])
    t_vt = din("vt", [P, KD, S])
    t_wq = din("wqt", [P, KD, D])
    t_wk = din("wkt", [P, KD, D])
    t_wv = din("wvt", [P, KD, D])
    t_wo = din("wot", [HD, H, KD, P])
    t_w1 = din("w1t", [P, KD, F])
    t_w2 = din("w2t", [P, FT, D])
    t_bq = din("bq", [P, KD], f32)
    t_bk = din("bk", [P, KD], f32)
    t_bv = din("bv_bc", [P, D], f32)
    t_bo = din("bo", [P, KD], f32)
    t_b1 = din("b1", [P, FT], f32)
    t_b2 = din("b2", [P, KD], f32)
    t_g1 = din("g1", [P, KD], f32)
    t_be1 = din("be1", [P, KD], f32)
    t_g2 = din("g2", [P, KD], f32)
    t_be2 = din("be2", [P, KD], f32)
    t_out = nc.dram_tensor("outT", [P, KD, SQ], f32, kind="ExternalOutput").ap()

    with tile.TileContext(nc) as tc, \
         tc.tile_pool(name="statics", bufs=1) as SP:
        def st(shape, dt, name):
            return SP.tile(shape, dt, tag=name, name=name)

        # ---- static SBUF tensors (DMAs emitted in first-use order) ----
        ones_bf = st([P, 1], bf, "ones_bf")
        nc.gpsimd.memset(ones_bf, 1.0 / D)
        eps_t = st([1, 1], f32, "eps_t")
        nc.gpsimd.memset(eps_t, EPS)
        warm_sq = st([1, 1], f32, "warm_sq")
        sink0 = st([1, 1], f32, "sink0")
        sink1 = st([1, 1], f32, "sink1")
        sink2 = st([1, 1], f32, "sink2")
        eps3 = st([1, 1], f32, "eps3")
        warm_rhs = st([1, SQ], bf, "warm_rhs")
        nc.gpsimd.memset(warm_rhs, 0.0)
        ones_row = st([P, HD], bf, "ones_row")
        nc.gpsimd.memset(ones_row, 1.0)
        eps2 = st([1, 1], f32, "eps2")

        bq = st([P, KD], f32, "bq")
        nc.sync.dma_start(out=bq, in_=t_bq)
        qbf = st([P, KD, SQ], bf, "qbf")
        nc.sync.dma_start(out=qbf, in_=t_qbf)
        wq = st([P, KD, D], bf, "wq")
        nc.sync.dma_start(out=wq, in_=t_wq)
        wk = st([P, KD, D], bf, "wk")
        nc.sync.dma_start(out=wk, in_=t_wk)
        kt_sb = st([P, KD, S], bf, "kt_sb")
        nc.sync.dma_start(out=kt_sb[:, :, 0:S // 2], in_=t_kt[:, :, 0:S // 2])
        bk = st([P, KD], f32, "bk")
        nc.sync.dma_start(out=bk, in_=t_bk)
        nc.sync.dma_start(out=kt_sb[:, :, S // 2:S], in_=t_kt[:, :, S // 2:S])
        wv = st([P, KD, D], bf, "wv")
        nc.sync.dma_start(out=wv, in_=t_wv)
        vt_sb = st([P, KD, S], bf, "vt_sb")
        nc.sync.dma_start(out=vt_sb[:, :, 0:S // 2], in_=t_vt[:, :, 0:S // 2])
        bv = st([P, D], f32, "bv")
        nc.sync.dma_start(out=bv, in_=t_bv)
        nc.sync.dma_start(out=vt_sb[:, :, S // 2:S], in_=t_vt[:, :, S // 2:S])
        bo = st([P, KD], f32, "bo")
        nc.sync.dma_start(out=bo, in_=t_bo)
        b1 = st([P, FT], f32, "b1")
        nc.sync.dma_start(out=b1, in_=t_b1)
        b2 = st([P, KD], f32, "b2")
        nc.sync.dma_start(out=b2, in_=t_b2)
        g1 = st([P, KD], f32, "g1")
        nc.sync.dma_start(out=g1, in_=t_g1)
        be1 = st([P, KD], f32, "be1")
        nc.sync.dma_start(out=be1, in_=t_be1)
        g2 = st([P, KD], f32, "g2")
        nc.sync.dma_start(out=g2, in_=t_g2)
        be2 = st([P, KD], f32, "be2")
        nc.sync.dma_start(out=be2, in_=t_be2)
        # activations kept for the whole kernel
        qh = st([P, KD, SQ], bf, "qh")
        kh = st([P, KD, S], bf, "kh")
        vh = st([P, KT, H, HD + 1], bf, "vh")
        avt = st([P, H, SQ], bf, "avt")
        xres = st([P, KD, SQ], f32, "xres")     # later reused as r2
        x1f = st([P, KD, SQ], f32, "x1f")
        x1b = st([P, KD, SQ], bf, "x1b")
        hsb = st([P, FT, SQ], bf, "hsb")
        qf32 = st([P, KD, SQ], f32, "qf32")     # later reused as LN2 out

        # late-phase statics: queued behind phase-1 inputs on purpose
        nc.sync.dma_start(out=qf32, in_=t_qf32)
        wo = st([P, H, KD, P], bf, "wo")
        nc.sync.dma_start(out=wo[0:HD], in_=t_wo)
        w1 = st([P, KD, F], bf, "w1")
        nc.sync.dma_start(out=w1, in_=t_w1)
        w2 = st([P, FT, D], bf, "w2")
        nc.sync.dma_start(out=w2, in_=t_w2)
        SP.seal()

        nc.gpsimd.memset(vh[:, :, :, HD:HD + 1], 1.0)

        # ---------------- phases 1+2: projections interleaved with attention
        # Emission order feeds the PE work for head-pair dt just before that
        # pair's attention, so projection matmuls fill the PE while ACT (exp)
        # is the binding engine. V-projection is folded into pair 0's key loop.
        bv8 = bv.rearrange("p (h d) -> p h d", h=H)

        with tc.tile_pool(name="att_sb", bufs=1) as asb, \
             tc.tile_pool(name="pj", bufs=2, space="PSUM") as pj, \
             tc.tile_pool(name="sc", bufs=1, space="PSUM") as scp, \
             tc.tile_pool(name="av", bufs=1, space="PSUM") as avp:

            def qk_groups(dt):
                """Projection matmul groups for head pair dt, each ~0.85us."""
                def qgroup():
                    ps = pj.tile([P, SQ], f32, tag="pj", name=f"psq{dt}")
                    for k in range(KD):
                        nc.tensor.matmul(ps, wq[:, k, dt * P:(dt + 1) * P],
                                         qbf[:, k, :], start=(k == 0),
                                         stop=(k == KD - 1))
                    nc.vector.tensor_scalar_add(qh[:, dt, :], ps, bq[:, dt:dt + 1])

                def kgroup(tb):
                    def go():
                        tbs = slice(tb * 512, (tb + 1) * 512)
                        ps = pj.tile([P, 512], f32, tag="pj", name=f"psk{dt}_{tb}")
                        for k in range(KD):
                            nc.tensor.matmul(ps, wk[:, k, dt * P:(dt + 1) * P],
                                             kt_sb[:, k, tbs],
                                             start=(k == 0), stop=(k == KD - 1))
                        nc.vector.tensor_scalar_add(kh[:, dt, tbs], ps,
                                                    bk[:, dt:dt + 1])
                    return go

                return [qgroup] + [kgroup(tb) for tb in range(TB)]

            def v_proj(tt):
                ps = pj.tile([P, D], f32, tag="pj", name=f"psv{tt}")
                for k in range(KD):
                    nc.tensor.matmul(ps, vt_sb[:, k, tt * P:(tt + 1) * P], wv[:, k, :],
                                     start=(k == 0), stop=(k == KD - 1))
                nc.vector.tensor_add(vh[:, tt, :, 0:HD],
                                     ps.rearrange("p (h d) -> p h d", h=H), bv8)

            # Wo runs as interleaved filler MMs inside pair 3 (heads 0-5 are
            # ready by then); po psum tiles live in the pj pool.
            po_tiles = {}

            def wo_mm(dt, h):
                def go():
                    if dt not in po_tiles:
                        po_tiles[dt] = pj.tile([P, SQ], f32, tag="pj",
                                               name=f"po{dt}")
                    nc.tensor.matmul(po_tiles[dt], wo[0:HD, h, dt, :],
                                     avt[0:HD, h, :], start=(h == 0),
                                     stop=(h == H - 1))
                return go

            # keep-warm matmuls: fill the initial DMA wait so the PE ramp is
            # warm when the real work lands. Anchored via the eps chain below.
            warm_ps = scp.tile([P, 2, SQ], f32, tag="sc0", bufs=1, name="warm_ps")
            for w in range(14):
                nc.tensor.matmul(warm_ps[0:1, 0, :], ones_bf[0:1, 0:1],
                                 warm_rhs, start=(w == 0), stop=(w == 13))
            nc.vector.tensor_scalar(out=sink0, in0=warm_ps[0:1, 0, 0:1],
                                    scalar1=0.0, scalar2=0.0,
                                    op0=mybir.AluOpType.mult,
                                    op1=mybir.AluOpType.add)

            # head: projections for pairs 0 and 1 (overlap the input DMAs)
            g0 = qk_groups(0)
            g0[0]()
            warm_ps2 = scp.tile([P, 2, SQ], f32, tag="sc1", bufs=1, name="warm_ps2")
            for w in range(10):
                nc.tensor.matmul(warm_ps2[0:1, 0, :], ones_bf[0:1, 0:1],
                                 warm_rhs, start=(w == 0), stop=(w == 9))
            nc.vector.tensor_scalar(out=sink1, in0=warm_ps2[0:1, 0, 0:1],
                                    scalar1=0.0, scalar2=0.0,
                                    op0=mybir.AluOpType.mult,
                                    op1=mybir.AluOpType.add)
            for g in g0[1:]:
                g()
            for g in qk_groups(1):
                g()
            fillers = []

            def normalize(hp_n, pav0_n, pav1_n):
                # partition_broadcast reads only partition 0 correctly on HW,
                # and the denominator lives at partition HD — broadcast it
                # with a K=1 PE matmul instead (reusing a score psum slot).
                for side, pav in ((0, pav0_n), (1, pav1_n)):
                    h = 2 * hp_n + side
                    rec = asb.tile([P, SQ], f32, tag="rec", bufs=2,
                                   name=f"rec{h}")
                    recb = asb.tile([P, SQ], bf, tag="recb", bufs=2,
                                    name=f"recb{h}")
                    nc.vector.reciprocal(rec[HD:HD + 1, :], pav[HD:HD + 1, :])
                    nc.vector.tensor_copy(recb[HD:HD + 1, :], rec[HD:HD + 1, :])
                    pbc = scp.tile([HD, SQ], f32, tag=f"sc{side}", bufs=1,
                                   name=f"pbc{h}")
                    nc.tensor.matmul(pbc, ones_row[HD:HD + 1, :],
                                     recb[HD:HD + 1, :], start=True, stop=True)
                    nc.vector.tensor_copy(rec[0:HD, :], pbc)
                    nc.vector.tensor_mul(avt[0:HD, h, :], pav[0:HD, :],
                                         rec[0:HD, :])

            prev_pavs = None
            for hp in range(KD):  # head pairs (2*hp, 2*hp+1)
                if hp == 1:
                    fillers += qk_groups(2)
                elif hp == 2:
                    fillers += qk_groups(3)
                elif hp == 3:
                    fillers += [wo_mm(dt, h) for dt in (0, 1) for h in range(6)]
                pav0 = avp.tile([P, SQ], f32, tag="av0")
                pav1 = avp.tile([P, SQ], f32, tag="av1")
                prev = None
                for k2 in range(KT // 2):  # pairs of key tiles
                    psc0 = scp.tile([P, 2, SQ], f32, tag="sc0", bufs=1)
                    psc1 = scp.tile([P, 2, SQ], f32, tag="sc1", bufs=1)
                    for i in range(2):
                        kt = 2 * k2 + i
                        ktl = slice(kt * P, (kt + 1) * P)
                        nc.tensor.matmul(psc0[:, i, :], kh[0:HD, hp, ktl],
                                         qh[0:HD, hp, :], start=True, stop=True)
                        nc.tensor.matmul(psc1[:, i, :], kh[HD:P, hp, ktl],
                                         qh[HD:P, hp, :], start=True, stop=True)
                    p0 = asb.tile([P, 2, SQ], bf, tag="p0", bufs=2)
                    nc.scalar.activation(out=p0, in_=psc0, func=ACT.Exp, scale=0.125)
                    p1 = asb.tile([P, 2, SQ], bf, tag="p1", bufs=2)
                    nc.scalar.activation(out=p1, in_=psc1, func=ACT.Exp, scale=0.125)
                    last_p1 = p1
                    if k2 == 0 and prev_pavs is not None:
                        normalize(*prev_pavs)
                        prev_pavs = None
                    if hp == 0:
                        v_proj(2 * k2)
                        v_proj(2 * k2 + 1)
                    elif fillers:
                        fillers.pop(0)()
                    if prev is not None:
                        q0, q1, pk2 = prev
                        for i in range(2):
                            kt = 2 * pk2 + i
                            nc.tensor.matmul(pav0[0:HD + 1, :], vh[:, kt, 2 * hp, :],
                                             q0[:, i, :], start=(kt == 0), stop=False)
                            nc.tensor.matmul(pav1[0:HD + 1, :],
                                             vh[:, kt, 2 * hp + 1, :],
                                             q1[:, i, :], start=(kt == 0), stop=False)
                    prev = (p0, p1, k2)
                q0, q1, pk2 = prev
                for i in range(2):
                    kt = 2 * pk2 + i
                    nc.tensor.matmul(pav0[0:HD + 1, :], vh[:, kt, 2 * hp, :],
                                     q0[:, i, :], start=False, stop=(kt == KT - 1))
                    nc.tensor.matmul(pav1[0:HD + 1, :], vh[:, kt, 2 * hp + 1, :],
                                     q1[:, i, :], start=False, stop=(kt == KT - 1))
                while hp == 3 and fillers:
                    fillers.pop(0)()
                prev_pavs = (hp, pav0, pav1)

            normalize(*prev_pavs)

            # Preload the sqrt table set in the idle ACT window between the
            # last exp and LN1; eps2 = warm * 0 + eps keeps the dependencies.
            nc.scalar.activation(out=warm_sq, in_=last_p1[0:1, 1, 0:1], func=ACT.Sqrt)
            nc.vector.tensor_add(warm_sq, warm_sq, sink0)
            nc.vector.tensor_add(warm_sq, warm_sq, sink1)
            nc.vector.tensor_scalar(out=eps2, in0=warm_sq, scalar1=0.0, scalar2=EPS,
                                    op0=mybir.AluOpType.mult, op1=mybir.AluOpType.add)

            # ------------ phase 3: finish Wo + residual ------------
            for dt in range(KD):
                for h in range(H):
                    if not (dt in (0, 1) and h < 6):
                        wo_mm(dt, h)()
                po = po_tiles[dt]
                nc.scalar.activation(out=po, in_=po, func=ACT.Identity,
                                     bias=bo[:, dt:dt + 1])
                nc.vector.tensor_add(xres[:, dt, :], po, qf32[:, dt, :])

        def layer_norm(src, gain, beta, dst_f32, dst_bf, stp, tmp, eps_ap):
            """dst = LN(src) * gain + beta over the partition (D) axis."""
            ps1 = stp.tile([1, SQ], f32, tag="s1")
            ps2 = stp.tile([1, SQ], f32, tag="s2")
            for dt in range(KD):
                xb = tmp.tile([P, SQ], bf, tag="xb", bufs=2)
                nc.vector.tensor_copy(xb, src[:, dt, :])
                sq = tmp.tile([P, SQ], bf, tag="sq", bufs=2)
                nc.vector.tensor_mul(sq, xb, xb)
                nc.tensor.matmul(ps1, ones_bf, xb, start=(dt == 0), stop=(dt == KD - 1))
                nc.tensor.matmul(ps2, ones_bf, sq, start=(dt == 0), stop=(dt == KD - 1))
            # ones_bf is 1/D, so ps1 = mean, ps2 = E[x^2] (x scaled by D elsewhere? no: rows)
            mean_sb = tmp.tile([1, SQ], f32, tag="ln_mean")
            nc.vector.tensor_copy(mean_sb, ps1)
            var = tmp.tile([1, SQ], f32, tag="ln_var")
            nc.vector.tensor_mul(var, mean_sb, mean_sb)
            nc.vector.tensor_sub(var, ps2, var)
            sd = tmp.tile([1, SQ], f32, tag="ln_sd")
            nc.scalar.activation(out=sd, in_=var, func=ACT.Sqrt, bias=eps_ap)
            rstd = tmp.tile([1, SQ], f32, tag="ln_rstd")
            nc.vector.reciprocal(rstd, sd)
            cvec = tmp.tile([1, SQ], f32, tag="ln_c")
            nc.vector.tensor_mul(cvec, mean_sb, rstd)
            pA = tmp.tile([P, SQ], f32, tag="bA")
            nc.gpsimd.partition_broadcast(pA, rstd)
            pC = tmp.tile([P, SQ], f32, tag="bC")
            nc.gpsimd.partition_broadcast(pC, cvec)
            for dt in range(KD):
                t1 = tmp.tile([P, SQ], f32, tag="t1", bufs=2)
                nc.vector.tensor_mul(t1, src[:, dt, :], pA)
                nc.vector.tensor_sub(t1, t1, pC)
                nc.scalar.activation(out=dst_f32[:, dt, :], in_=t1, func=ACT.Identity,
                                     bias=beta[:, dt:dt + 1], scale=gain[:, dt:dt + 1])
                if dst_bf is not None:
                    nc.vector.tensor_copy(dst_bf[:, dt, :], dst_f32[:, dt, :])

        with tc.tile_pool(name="ln1_sb", bufs=1) as tmp1, \
             tc.tile_pool(name="st1", bufs=1, space="PSUM") as stp1, \
             tc.tile_pool(name="wm1", bufs=1, space="PSUM") as wmp1:
            # keep the PE ramp warm across the LN1 serial chain
            warm1 = wmp1.tile([1, SQ], f32, tag="wm", name="warm1")
            for w in range(12):
                nc.tensor.matmul(warm1, ones_bf[0:1, 0:1], warm_rhs,
                                 start=(w == 0), stop=(w == 11))
            layer_norm(xres, g1, be1, x1f, x1b, stp1, tmp1, eps2)
            warm1b = wmp1.tile([1, SQ], f32, tag="wm2", name="warm1b")
            for w in range(16):
                nc.tensor.matmul(warm1b, ones_bf[0:1, 0:1], warm_rhs,
                                 start=(w == 0), stop=(w == 15))
            nc.vector.tensor_scalar(out=sink2, in0=warm1[0:1, 0:1], scalar1=0.0,
                                    scalar2=0.0, op0=mybir.AluOpType.mult,
                                    op1=mybir.AluOpType.add)
            nc.vector.tensor_scalar(out=eps3, in0=warm1b[0:1, 0:1], scalar1=0.0,
                                    scalar2=EPS, op0=mybir.AluOpType.mult,
                                    op1=mybir.AluOpType.add)
            nc.vector.tensor_add(eps3, eps3, sink2)

        # ---------------- phase 4: FFN ----------------
        with tc.tile_pool(name="pf", bufs=5, space="PSUM") as pfp:
            for ft in range(FT):
                pf = pfp.tile([P, SQ], f32, tag="pf")
                for k in range(KD):
                    nc.tensor.matmul(pf, w1[:, k, ft * P:(ft + 1) * P], x1b[:, k, :],
                                     start=(k == 0), stop=(k == KD - 1))
                nc.scalar.activation(out=hsb[:, ft, :], in_=pf, func=ACT.Relu,
                                     bias=b1[:, ft:ft + 1])

        r2 = xres      # dead after LN1 -> reuse for x1 + ffn
        outsb = qf32   # dead after the Wo residual add -> reuse for LN2 out
        with tc.tile_pool(name="ln2_sb", bufs=1) as tmp2, \
             tc.tile_pool(name="py", bufs=3, space="PSUM") as pyp, \
             tc.tile_pool(name="st2", bufs=1, space="PSUM") as stp2:
            for dt in range(KD):
                py = pyp.tile([P, SQ], f32, tag="py")
                for ft in range(FT):
                    nc.tensor.matmul(py, w2[:, ft, dt * P:(dt + 1) * P], hsb[:, ft, :],
                                     start=(ft == 0), stop=(ft == FT - 1))
                nc.scalar.activation(out=py, in_=py, func=ACT.Identity,
                                     bias=b2[:, dt:dt + 1])
                nc.vector.tensor_add(r2[:, dt, :], py, x1f[:, dt, :])
            layer_norm(r2, g2, be2, outsb, None, stp2, tmp2, eps3)
            for dt in range(KD):
                nc.sync.dma_start(out=t_out[:, dt, :], in_=outsb[:, dt, :])

    nc.compile()
    return nc


def _get_nc():
    if "nc" not in _CACHE:
        _CACHE["nc"] = _build_nc()
    return _CACHE["nc"]


def make_in_maps(q, k, v, Wq, bq, Wk, bk, Wv, bv, Wo, bo, W1, b1, W2, b2,
                 g1, be1, g2, be2):
    f32 = np.float32

    def tile_pd(x, n):  # [n*P] -> [P, n]
        return np.ascontiguousarray(np.asarray(x, f32).reshape(n, P).T)

    def wt(w, cols):  # [in, out] -> [P, in//P, out]
        return np.ascontiguousarray(
            np.asarray(w, f32).T.reshape(-1, P, cols).transpose(1, 0, 2)).astype(BF16)

    shared = {
        "wqt": wt(Wq, D), "wkt": wt(Wk, D), "wvt": wt(Wv, D),
        "w1t": wt(W1, F), "w2t": wt(W2, D),
        "wot": np.ascontiguousarray(
            np.asarray(Wo, f32).T.reshape(H, HD, KD, P).transpose(1, 0, 2, 3)
        ).astype(BF16),
        "bq": tile_pd(bq, KD), "bk": tile_pd(bk, KD),
        "bv_bc": np.ascontiguousarray(
            np.broadcast_to(np.asarray(bv, f32), (P, D))),
        "bo": tile_pd(bo, KD), "b1": tile_pd(b1, FT), "b2": tile_pd(b2, KD),
        "g1": tile_pd(g1, KD), "be1": tile_pd(be1, KD),
        "g2": tile_pd(g2, KD), "be2": tile_pd(be2, KD),
    }

    q = np.asarray(q, f32)
    k = np.asarray(k, f32)
    v = np.asarray(v, f32)
    def fm(x):  # [S, D] -> [P, KD, S] feature-major partition-contiguous
        return np.ascontiguousarray(
            x.T.reshape(KD, P, S).transpose(1, 0, 2)).astype(BF16)

    kts = [fm(k[b]) for b in range(B)]
    vts = [fm(v[b]) for b in range(B)]

    in_maps = []
    for c in range(NCORES):
        b, s0 = c // 4, (c % 4) * SQ
        qt = np.ascontiguousarray(q[b, s0:s0 + SQ, :].T)          # [D, SQ]
        qt4 = np.ascontiguousarray(qt.reshape(KD, P, SQ).transpose(1, 0, 2))
        in_maps.append({
            "qt_bf": qt4.astype(BF16), "qt_f32": qt4,
            "kt": kts[b], "vt": vts[b], **shared,
        })
    return in_maps


def assemble_out(results):
    out = np.empty((B, S, D), np.float32)
    for c in range(NCORES):
        b, s0 = c // 4, (c % 4) * SQ
        # outT [P, KD, SQ]: feature dt*P+p, token t -> out[t, feature]
        out[b, s0:s0 + SQ, :] = results[c]["outT"].transpose(2, 1, 0).reshape(SQ, D)
    return out


def kernel(**inputs):
    global LAST_RESULT
    import os

    from concourse.bass_utils import run_bass_kernel_spmd

    nc = _get_nc()
    in_maps = make_in_maps(**inputs)
    try:
        res = run_bass_kernel_spmd(nc, in_maps, core_ids=list(range(NCORES)))
    except ModuleNotFoundError:
        # BASS_TRACE set but this container has no axon NTFF profile hook
        # (antenv.axon_hooks missing) — rerun untraced.
        os.environ["BASS_NEVER_TRACE"] = "1"
        res = run_bass_kernel_spmd(nc, in_maps, core_ids=list(range(NCORES)))
    LAST_RESULT = res
    return assemble_out(res.results)



# revision 9
# speedup vs baseline: 1.1087x; 1.1087x over previous
"""Trainium2 Bass kernel for a post-norm transformer encoder layer.

Contract: kernel(**inputs) takes the FULL fp32 inputs (as produced by the
problem's setup_inputs) and returns the FULL [2, 2048, 512] fp32 output.

Sharding (8 cores, no collectives): core c owns 512 query tokens of batch
c // 4 (slice (c % 4) * 512). Each core recomputes the K/V projections for
its whole batch (2048 tokens) and runs attention + FFN for its 512 queries.

Fast path: every GEMM runs as fp8e4 DoubleRow matmuls (2 contraction tiles
per instruction at 0.5 cycles/row). Weights are host-scaled by 64 into the
fp8 normal range; every psum consumer applies the inverse power-of-two
scale for free inside the op it already needed. Softmax exp is split
between the ACT engine (Exp) and gpsimd (pow with base e^(1/128)); all
other ACT work is folded away (biases into host-precomputed vectors, LN
sqrt via gpsimd pow). Post-attention arithmetic is bf16 end to end.
"""

import numpy as np
import ml_dtypes

D = 512
S = 2048
B = 2
H = 8
HD = 64
F = 2048
EPS = 1e-5
NCORES = 8
SQ = 512          # queries per core
P = 128           # partitions
KD = D // P       # 4   D-tiles
KT = S // P       # 16  key tiles
TB = S // 512     # 4   512-token blocks
FT = F // P       # 16  FFN hidden tiles
HP = H // 2       # 4   head pairs
K2 = KT // 2      # 8   key-tile pairs
VC = 96           # padded AV columns: 64 values + 1 ones + 31 zeros

WS = 64.0         # host weight scale
SCALE_QKV = 1.0 / 16.0    # psum *  -> activations stored x4
SCALE_WO = 2.0 ** -12
SCALE_FF2 = 2.0 ** -8
ONES_COL = 1.0 / 16.0     # vh ones column value -> avt = 64*av
EXP_SCALE = 1.0 / 128.0   # scores psum = 16 * true score; softmax /8

BF16 = ml_dtypes.bfloat16
F8 = ml_dtypes.float8_e4m3

# k2 indices whose head-1 exp runs on gpsimd (via DVE psum->sbuf copy)
POOL_K2 = (1, 3, 5)
# FFN1 consume engines per ft tile: 'a' = ACT relu, 'd' = DVE
# (gpsimd cannot read PSUM, so no Pool here)
FT_ENG = "aaaaaaaadddddddd"

_CACHE = {}
LAST_RESULT = None


def _build_nc():
    import concourse.bacc as bacc
    import concourse.tile as tile
    from concourse import mybir

    bf = mybir.dt.bfloat16
    f8 = mybir.dt.float8e4
    f32 = mybir.dt.float32
    ACT = mybir.ActivationFunctionType
    ALU = mybir.AluOpType
    DR = mybir.MatmulPerfMode.DoubleRow

    nc = bacc.Bacc("TRN2", target_bir_lowering=False, debug=False)

    def din(name, shape, dt=f8):
        return nc.dram_tensor(name, shape, dt, kind="ExternalInput").ap()

    t_q8 = din("q8", [P, KD, SQ])
    t_qbf = din("qbf16", [P, KD, SQ], bf)
    t_kt = din("kt8", [P, KD, S])
    t_vt = din("vt8", [P, KD, S])
    t_wq = din("wq8", [P, KD, D])
    t_wk = din("wk8", [P, KD, D])
    t_wv = din("wv8", [P, KD, D])
    t_wo = din("wo8", [HD, HP, 2, KD, P])
    t_w1 = din("w18", [P, KD, F])
    t_w2 = din("w28", [P, FT, D])
    t_zed = din("zed8", [P, 4096])
    t_bq = din("bq4", [P, KD], f32)
    t_bk = din("bk4", [P, KD], f32)
    t_b1m = din("b1m64", [P, FT], f32)
    t_b1p = din("b1p64", [P, FT], f32)
    t_g1 = din("g1", [P, KD], f32)
    t_be1 = din("be1", [P, KD], f32)
    t_be1p = din("be1p", [P, KD], f32)
    t_g2 = din("g2", [P, KD], f32)
    t_be2 = din("be2", [P, KD], f32)
    t_out = nc.dram_tensor("outT", [P, KD, SQ], bf, kind="ExternalOutput").ap()

    with tile.TileContext(nc) as tc, \
         tc.tile_pool(name="statics", bufs=1) as SP:
        def st(shape, dt, name):
            return SP.tile(shape, dt, tag=name, name=name)

        ones_bf = st([P, 1], bf, "ones_bf")
        nc.gpsimd.memset(ones_bf, 1.0 / D)
        warm_rhs = st([1, SQ], bf, "warm_rhs")
        nc.gpsimd.memset(warm_rhs, 0.0)
        ones_row = st([P, HD], bf, "ones_row")
        nc.gpsimd.memset(ones_row, 1.0)
        ebase = st([P, 2, SQ], bf, "ebase")
        nc.gpsimd.memset(ebase, float(np.exp(EXP_SCALE)))
        half_t = st([1, SQ], f32, "half_t")
        nc.gpsimd.memset(half_t, 0.5)
        sink0 = st([1, 1], f32, "sink0")
        sink1 = st([1, 1], f32, "sink1")

        # ---- DMAs in first-use order ----
        bq = st([P, KD], f32, "bq")
        nc.sync.dma_start(out=bq, in_=t_bq)
        q8 = st([P, KD, SQ], f8, "q8")
        nc.sync.dma_start(out=q8, in_=t_q8)
        wq = st([P, KD, D], f8, "wq")
        nc.sync.dma_start(out=wq, in_=t_wq)
        wk = st([P, KD, D], f8, "wk")
        nc.sync.dma_start(out=wk, in_=t_wk)
        bk = st([P, KD], f32, "bk")
        nc.sync.dma_start(out=bk, in_=t_bk)
        kt_sb = st([P, KD, S + P], f8, "kt_sb")     # K proj moving data
        nc.sync.dma_start(out=kt_sb[:, :, 0:S // 2], in_=t_kt[:, :, 0:S // 2])

        # persistent activations
        q_z = st([P, KD, 2, SQ], f8, "q_z")         # slot1 = zeros
        kh = st([P, KD, S + P], f8, "kh")           # +128 zero pad for DR dup
        vh = st([P, KT, H, VC], f8, "vh")
        avt = st([P, H, SQ], f8, "avt")
        xres = st([P, KD, SQ], bf, "xres")          # residual; reused as r2
        x1b = st([P, KD, SQ], f8, "x1b")
        x1f = st([P, KD, SQ], bf, "x1f")
        hsb = st([P, FT, SQ], f8, "hsb")
        outsb = st([P, KD, SQ], bf, "outsb")

        nc.sync.dma_start(out=q_z[:, :, 1, :], in_=t_zed[:, 0:KD * SQ].rearrange(
            "p (k s) -> p k s", k=KD))
        nc.sync.dma_start(out=kh[:, :, S:S + P], in_=t_zed[:, 0:KD * P].rearrange(
            "p (k s) -> p k s", k=KD))
        nc.sync.dma_start(out=kt_sb[:, :, S // 2:S], in_=t_kt[:, :, S // 2:S])
        wv = st([P, KD, D], f8, "wv")
        nc.sync.dma_start(out=wv, in_=t_wv)
        vt_sb = st([P, KD, S], f8, "vt_sb")
        nc.sync.dma_start(out=vt_sb[:, :, 0:S // 2], in_=t_vt[:, :, 0:S // 2])
        nc.sync.dma_start(out=vh[:, :, :, HD + 1:VC], in_=t_zed[:, 0:KT * H * 31].rearrange(
            "p (t h c) -> p t h c", t=KT, h=H))
        nc.sync.dma_start(out=vt_sb[:, :, S // 2:S], in_=t_vt[:, :, S // 2:S])
        # tail-phase inputs, queued last
        qbf16 = st([P, KD, SQ], bf, "qbf16")
        nc.sync.dma_start(out=qbf16, in_=t_qbf)
        wo = st([HD, HP, 2, KD, P], f8, "wo")
        nc.sync.dma_start(out=wo, in_=t_wo)
        w1 = st([P, KD, F], f8, "w1")
        nc.sync.dma_start(out=w1, in_=t_w1)
        w2 = st([P, FT, D], f8, "w2")
        nc.sync.dma_start(out=w2, in_=t_w2)
        b1m = st([P, FT], f32, "b1m")
        nc.sync.dma_start(out=b1m, in_=t_b1m)
        b1p = st([P, FT], f32, "b1p")
        nc.sync.dma_start(out=b1p, in_=t_b1p)
        g1 = st([P, KD], f32, "g1")
        nc.sync.dma_start(out=g1, in_=t_g1)
        be1 = st([P, KD], f32, "be1")
        nc.sync.dma_start(out=be1, in_=t_be1)
        be1p = st([P, KD], f32, "be1p")
        nc.sync.dma_start(out=be1p, in_=t_be1p)
        g2 = st([P, KD], f32, "g2")
        nc.sync.dma_start(out=g2, in_=t_g2)
        be2 = st([P, KD], f32, "be2")
        nc.sync.dma_start(out=be2, in_=t_be2)
        SP.seal()

        nc.gpsimd.memset(vh[:, :, :, HD:HD + 1], ONES_COL)

        # ============ phases 1+2: projections interleaved with attention
        with tc.tile_pool(name="att_sb", bufs=1) as asb, \
             tc.tile_pool(name="pj", bufs=2, space="PSUM") as pj, \
             tc.tile_pool(name="sc", bufs=1, space="PSUM") as scp, \
             tc.tile_pool(name="av", bufs=1, space="PSUM") as avp:

            def qgroup(dt):
                ps = pj.tile([P, SQ], f32, tag="pj", name=f"psq{dt}")
                for k in (0, 2):
                    nc.tensor.matmul(ps, wq[:, k:k + 2, dt * P:(dt + 1) * P],
                                     q8[:, k:k + 2, :], start=(k == 0),
                                     stop=(k == 2), perf_mode=DR)
                nc.vector.tensor_scalar(out=q_z[:, dt, 0, :], in0=ps,
                                        scalar1=SCALE_QKV, scalar2=bq[:, dt:dt + 1],
                                        op0=ALU.mult, op1=ALU.add)

            def kgroup(dt, tb):
                def go():
                    tbs = slice(tb * 512, (tb + 1) * 512)
                    ps = pj.tile([P, 512], f32, tag="pj", name=f"psk{dt}_{tb}")
                    for k in (0, 2):
                        nc.tensor.matmul(ps, wk[:, k:k + 2, dt * P:(dt + 1) * P],
                                         kt_sb[:, k:k + 2, tbs], start=(k == 0),
                                         stop=(k == 2), perf_mode=DR)
                    nc.vector.tensor_scalar(out=kh[:, dt, tbs], in0=ps,
                                            scalar1=SCALE_QKV,
                                            scalar2=bk[:, dt:dt + 1],
                                            op0=ALU.mult, op1=ALU.add)
                return go

            def qk_groups(dt):
                return [lambda: qgroup(dt)] + [kgroup(dt, tb) for tb in range(TB)]

            def v_proj(tt):
                ps = pj.tile([P, D], f32, tag="pj", name=f"psv{tt}")
                for k in (0, 2):
                    nc.tensor.matmul(ps, vt_sb[:, k:k + 2, tt * P:(tt + 1) * P],
                                     wv[:, k:k + 2, :], start=(k == 0),
                                     stop=(k == 2), perf_mode=DR)
                nc.vector.tensor_scalar_mul(
                    vh[:, tt, :, 0:HD], ps.rearrange("p (h d) -> p h d", h=H),
                    SCALE_QKV)

            # Wo chains: po psum tiles live in the pj pool (projections are
            # done by the time these run).
            po_tiles = {}

            def wo_mm(dt, hp_i):
                def go():
                    if dt not in po_tiles:
                        po_tiles[dt] = pj.tile([P, SQ], f32, tag="pj",
                                               name=f"po{dt}")
                    nc.tensor.matmul(po_tiles[dt], wo[:, hp_i, :, dt, :],
                                     avt[0:HD, 2 * hp_i:2 * hp_i + 2, :],
                                     start=(hp_i == 0), stop=(hp_i == HP - 1),
                                     perf_mode=DR)
                return go

            def wo_consume(dt):
                po = po_tiles.pop(dt)
                nc.vector.scalar_tensor_tensor(
                    out=xres[:, dt, :], in0=po, scalar=SCALE_WO,
                    in1=qbf16[:, dt, :], op0=ALU.mult, op1=ALU.add)

            # keep-warm matmuls to cover the initial DMA wait
            warm_ps = pj.tile([P, SQ], f32, tag="pj", name="warm_ps")
            for w in range(14):
                nc.tensor.matmul(warm_ps[0:1, :], ones_bf[0:1, 0:1], warm_rhs,
                                 start=(w == 0), stop=(w == 13))
            nc.vector.tensor_scalar(out=sink0, in0=warm_ps[0:1, 0:1],
                                    scalar1=0.0, scalar2=0.0,
                                    op0=ALU.mult, op1=ALU.add)

            qgroup(0)
            for g in [kgroup(0, tb) for tb in range(TB)]:
                g()
            for g in qk_groups(1):
                g()
            fillers = []

            def normalize(hp_n, pav0_n, pav1_n):
                for side, pav in ((0, pav0_n), (1, pav1_n)):
                    h = 2 * hp_n + side
                    rec = asb.tile([1, SQ], bf, tag="rec", bufs=2,
                                   name=f"rec{h}")
                    with nc.allow_low_precision(reason="softmax denom, bf16 ok"):
                        nc.vector.reciprocal(rec, pav[HD:HD + 1, :])
                    rbc = asb.tile([HD, SQ], bf, tag="rbc", bufs=2,
                                   name=f"rbc{h}")
                    nc.gpsimd.partition_broadcast(rbc, rec)
                    nc.vector.tensor_mul(avt[0:HD, h, :], pav[0:HD, :], rbc)

            prev_pavs = None
            for hp in range(HP):
                if hp == 1:
                    fillers += qk_groups(2)
                elif hp == 2:
                    fillers += qk_groups(3)
                elif hp == 3:
                    fillers += [wo_mm(dt, i) for i in range(3) for dt in (0, 1)]
                pav0 = avp.tile([P, SQ], f32, tag="av0")
                pav1 = avp.tile([P, SQ], f32, tag="av1")
                prev = None
                for k2 in range(K2):
                    psc = scp.tile([P, 2, 2, SQ], f32, tag="sc", bufs=1)
                    for side in range(2):
                        pr = slice(side * HD, side * HD + HD)
                        for i in range(2):
                            kt = 2 * k2 + i
                            lw = kh[pr, hp, kt * P:(kt + 2) * P].rearrange(
                                "p (two k) -> p two k", two=2)
                            nc.tensor.matmul(psc[:, side, i, :], lw,
                                             q_z[pr, hp, :, :], start=True,
                                             stop=True, perf_mode=DR)
                    p = asb.tile([P, 2, 2, SQ], f8, tag="p", bufs=2)
                    if k2 in POOL_K2:
                        nc.scalar.activation(out=p[:, 0, :, :], in_=psc[:, 0, :, :],
                                             func=ACT.Exp, scale=EXP_SCALE)
                        scb = asb.tile([P, 2, SQ], bf, tag="scb", bufs=2)
                        nc.vector.tensor_copy(scb, psc[:, 1, :, :])
                        nc.gpsimd.tensor_tensor(p[:, 1, :, :], ebase, scb, ALU.pow)
                    else:
                        nc.scalar.activation(out=p, in_=psc, func=ACT.Exp,
                                             scale=EXP_SCALE)
                    if k2 == 0 and prev_pavs is not None:
                        normalize(*prev_pavs)
                        prev_pavs = None
                    if hp == 0:
                        v_proj(2 * k2)
                        v_proj(2 * k2 + 1)
                    elif fillers:
                        fillers.pop(0)()
                    if prev is not None:
                        pp, pk2 = prev
                        for side, pav in ((0, pav0), (1, pav1)):
                            nc.tensor.matmul(
                                pav[0:VC, :],
                                vh[:, 2 * pk2:2 * pk2 + 2, 2 * hp + side, :],
                                pp[:, side, :, :], start=(pk2 == 0),
                                stop=False, perf_mode=DR)
                    prev = (p, k2)
                pp, pk2 = prev
                for side, pav in ((0, pav0), (1, pav1)):
                    nc.tensor.matmul(
                        pav[0:VC, :], vh[:, 2 * pk2:2 * pk2 + 2, 2 * hp + side, :],
                        pp[:, side, :, :], start=False, stop=True, perf_mode=DR)
                while hp == 3 and fillers:
                    fillers.pop(0)()
                prev_pavs = (hp, pav0, pav1)

            normalize(*prev_pavs)

            # finish Wo for dt 0/1 (head pair 3) + residual
            for dt in (0, 1):
                wo_mm(dt, 3)()
                wo_consume(dt)

        # ============ phase 3: Wo tail + LN1 ============
        def layer_norm(src, gain, outs, stp, tmp, wmp, nwarm):
            """outs: list of (dst, beta_ap, dtype-handled-by-dst)."""
            warm = wmp.tile([P, SQ], f32, tag="wm", name="lnwarm")
            for w in range(nwarm):
                nc.tensor.matmul(warm[0:1, :], ones_bf[0:1, 0:1], warm_rhs,
                                 start=(w == 0), stop=(w == nwarm - 1))
            ps1 = stp.tile([1, SQ], f32, tag="s1")
            ps2 = stp.tile([1, SQ], f32, tag="s2")
            for dt in range(KD):
                sq = tmp.tile([P, SQ], bf, tag="sq", bufs=2)
                nc.vector.tensor_mul(sq, src[:, dt, :], src[:, dt, :])
                nc.tensor.matmul(ps1, ones_bf, src[:, dt, :], start=(dt == 0),
                                 stop=(dt == KD - 1))
                nc.tensor.matmul(ps2, ones_bf, sq, start=(dt == 0),
                                 stop=(dt == KD - 1))
            mean = tmp.tile([1, SQ], f32, tag="ln_mean")
            nc.vector.tensor_copy(mean, ps1)
            m2 = tmp.tile([1, SQ], f32, tag="ln_m2")
            nc.vector.tensor_mul(m2, mean, mean)
            m2e = tmp.tile([1, SQ], f32, tag="ln_m2e")
            nc.vector.tensor_scalar(out=m2e, in0=m2, scalar1=-1.0, scalar2=EPS,
                                    op0=ALU.mult, op1=ALU.add)
            var = tmp.tile([1, SQ], f32, tag="ln_var")
            nc.vector.tensor_add(var, ps2, m2e)
            sd = tmp.tile([1, SQ], f32, tag="ln_sd")
            nc.gpsimd.tensor_tensor(sd, var, half_t, ALU.pow)
            rstd = tmp.tile([1, SQ], bf, tag="ln_rstd")
            with nc.allow_low_precision(reason="LN rstd, bf16 ok"):
                nc.vector.reciprocal(rstd, sd)
            cvec = tmp.tile([1, SQ], bf, tag="ln_c")
            nc.vector.tensor_mul(cvec, mean, rstd)
            pA = tmp.tile([P, SQ], bf, tag="bA")
            nc.gpsimd.partition_broadcast(pA, rstd)
            pC = tmp.tile([P, SQ], bf, tag="bC")
            nc.gpsimd.partition_broadcast(pC, cvec)
            nc.vector.tensor_scalar(out=sink1, in0=warm[0:1, 0:1], scalar1=0.0,
                                    scalar2=0.0, op0=ALU.mult, op1=ALU.add)
            for dt in range(KD):
                t1 = tmp.tile([P, SQ], bf, tag="t1", bufs=2)
                nc.vector.tensor_mul(t1, src[:, dt, :], pA)
                nc.vector.tensor_sub(t1, t1, pC)
                for dst, beta in outs:
                    nc.vector.tensor_scalar(out=dst[:, dt, :], in0=t1,
                                            scalar1=gain[:, dt:dt + 1],
                                            scalar2=beta[:, dt:dt + 1],
                                            op0=ALU.mult, op1=ALU.add)

        with tc.tile_pool(name="ln1_sb", bufs=1) as tmp1, \
             tc.tile_pool(name="po2", bufs=2, space="PSUM") as pop, \
             tc.tile_pool(name="st1", bufs=1, space="PSUM") as stp1, \
             tc.tile_pool(name="wm1", bufs=1, space="PSUM") as wmp1:
            for dt in (2, 3):
                po_tiles[dt] = pop.tile([P, SQ], f32, tag="po", name=f"po{dt}")
                for hp_i in range(HP):
                    nc.tensor.matmul(po_tiles[dt], wo[:, hp_i, :, dt, :],
                                     avt[0:HD, 2 * hp_i:2 * hp_i + 2, :],
                                     start=(hp_i == 0), stop=(hp_i == HP - 1),
                                     perf_mode=DR)
                wo_consume(dt)
            layer_norm(xres, g1, [(x1b, be1), (x1f, be1p)], stp1, tmp1, wmp1, 22)

        # ============ phase 4: FFN ============
        with tc.tile_pool(name="pf", bufs=4, space="PSUM") as pfp, \
             tc.tile_pool(name="py", bufs=2, space="PSUM") as pyp, \
             tc.tile_pool(name="wm2", bufs=1, space="PSUM") as wmp2:
            for ft in range(FT):
                pf = pfp.tile([P, SQ], f32, tag="pf")
                for k in (0, 2):
                    nc.tensor.matmul(pf, w1[:, k:k + 2, ft * P:(ft + 1) * P],
                                     x1b[:, k:k + 2, :], start=(k == 0),
                                     stop=(k == 2), perf_mode=DR)
                eng = FT_ENG[ft]
                if eng == "a":
                    nc.scalar.activation(out=hsb[:, ft, :], in_=pf, func=ACT.Relu,
                                         bias=b1p[:, ft:ft + 1])
                else:
                    e = nc.vector if eng == "d" else nc.gpsimd
                    e.tensor_scalar(out=hsb[:, ft, :], in0=pf,
                                    scalar1=b1m[:, ft:ft + 1], scalar2=SCALE_QKV,
                                    op0=ALU.max, op1=ALU.mult)
            for dt in range(KD):
                py = pyp.tile([P, SQ], f32, tag="py")
                for f in range(0, FT, 2):
                    nc.tensor.matmul(py, w2[:, f:f + 2, dt * P:(dt + 1) * P],
                                     hsb[:, f:f + 2, :], start=(f == 0),
                                     stop=(f == FT - 2), perf_mode=DR)
                nc.vector.scalar_tensor_tensor(
                    out=xres[:, dt, :], in0=py, scalar=SCALE_FF2,
                    in1=x1f[:, dt, :], op0=ALU.mult, op1=ALU.add)

        # ============ phase 5: LN2 + output ============
        with tc.tile_pool(name="ln2_sb", bufs=1) as tmp2, \
             tc.tile_pool(name="st2", bufs=1, space="PSUM") as stp2, \
             tc.tile_pool(name="wm3", bufs=1, space="PSUM") as wmp3:
            layer_norm(xres, g2, [(outsb, be2)], stp2, tmp2, wmp3, 22)
            for dt in range(KD):
                nc.sync.dma_start(out=t_out[:, dt, :], in_=outsb[:, dt, :])

    nc.compile()
    return nc


def _get_nc():
    if "nc" not in _CACHE:
        _CACHE["nc"] = _build_nc()
    return _CACHE["nc"]


def make_in_maps(q, k, v, Wq, bq, Wk, bk, Wv, bv, Wo, bo, W1, b1, W2, b2,
                 g1, be1, g2, be2):
    f32 = np.float32
    q = np.asarray(q, f32)
    k = np.asarray(k, f32)
    v = np.asarray(v, f32)
    Wq, Wk, Wv, Wo = (np.asarray(x, f32) for x in (Wq, Wk, Wv, Wo))
    W1, W2 = np.asarray(W1, f32), np.asarray(W2, f32)
    bq, bk, bv, bo = (np.asarray(x, f32) for x in (bq, bk, bv, bo))
    b1, b2 = np.asarray(b1, f32), np.asarray(b2, f32)
    g1, be1, g2, be2 = (np.asarray(x, f32) for x in (g1, be1, g2, be2))

    def tile_pd(x, n):  # [n*P] -> [P, n]
        return np.ascontiguousarray(x.reshape(n, P).T)

    def wt8(w, cols):  # [out, in] -> [P, in//P, cols] scaled fp8
        return np.ascontiguousarray(
            (w.T * WS).reshape(-1, P, cols).transpose(1, 0, 2)).astype(F8)

    # per-row (hidden-unit) scale for W2: ACT-relu rows hold 64h -> x4,
    # max-trick rows hold 4t -> x64
    row_scale = np.empty(F, f32)
    t_mask = np.zeros(F, bool)
    for ft in range(FT):
        sl = slice(ft * P, (ft + 1) * P)
        if FT_ENG[ft] == "a":
            row_scale[sl] = 4.0
        else:
            row_scale[sl] = 64.0
            t_mask[sl] = True
    w28 = np.ascontiguousarray(
        (W2.T * row_scale[:, None]).reshape(FT, P, D).transpose(1, 0, 2)).astype(F8)

    bo_eff = bo + Wo @ bv
    b2_eff = b2 + W2[:, t_mask] @ b1[t_mask]

    wo8 = np.ascontiguousarray(
        (Wo.T * WS).reshape(HP, 2, HD, KD, P).transpose(2, 0, 1, 3, 4)).astype(F8)

    shared = {
        "wq8": wt8(Wq, D), "wk8": wt8(Wk, D), "wv8": wt8(Wv, D),
        "w18": wt8(W1, F), "w28": w28, "wo8": wo8,
        "zed8": np.zeros((P, 4096), F8),
        "bq4": tile_pd(4.0 * bq, KD), "bk4": tile_pd(4.0 * bk, KD),
        "b1m64": tile_pd(-64.0 * b1, FT), "b1p64": tile_pd(64.0 * b1, FT),
        "g1": tile_pd(g1, KD), "be1": tile_pd(be1, KD),
        "be1p": tile_pd(be1 + b2_eff, KD),
        "g2": tile_pd(g2, KD), "be2": tile_pd(be2, KD),
    }

    def fm(x, dt):  # [S, D] -> [P, KD, S] feature-major
        return np.ascontiguousarray(
            x.T.reshape(KD, P, -1).transpose(1, 0, 2)).astype(dt)

    kts = [fm(k[b], F8) for b in range(B)]
    vts = [fm(v[b], F8) for b in range(B)]

    in_maps = []
    for c in range(NCORES):
        b, s0 = c // 4, (c % 4) * SQ
        qs = q[b, s0:s0 + SQ, :]
        in_maps.append({
            "q8": fm(qs, F8),
            "qbf16": fm(qs + bo_eff, BF16),
            "kt8": kts[b], "vt8": vts[b], **shared,
        })
    return in_maps


def assemble_out(results):
    out = np.empty((B, S, D), np.float32)
    for c in range(NCORES):
        b, s0 = c // 4, (c % 4) * SQ
        out[b, s0:s0 + SQ, :] = results[c]["outT"].astype(np.float32) \
            .transpose(2, 1, 0).reshape(SQ, D)
    return out


def kernel(**inputs):
    global LAST_RESULT
    import os

    from concourse.bass_utils import run_bass_kernel_spmd

    nc = _get_nc()
    in_maps = make_in_maps(**inputs)
    try:
        res = run_bass_kernel_spmd(nc, in_maps, core_ids=list(range(NCORES)))
    except ModuleNotFoundError:
        # BASS_TRACE set but this container has no axon NTFF profile hook
        # (antenv.axon_hooks missing) — rerun untraced.
        os.environ["BASS_NEVER_TRACE"] = "1"
        res = run_bass_kernel_spmd(nc, in_maps, core_ids=list(range(NCORES)))
    LAST_RESULT = res
    return assemble_out(res.results)


# revision 16
# speedup vs baseline: 1.4117x; 1.2733x over previous
"""Trainium2 Bass kernel for a post-norm transformer encoder layer.

Contract: kernel(**inputs) takes the FULL fp32 inputs (as produced by the
problem's setup_inputs) and returns the FULL [2, 2048, 512] fp32 output.

Sharding (8 cores, no collectives): core c owns 512 query tokens of batch
c // 4 (slice (c % 4) * 512). Each core recomputes the K/V projections for
its whole batch (2048 tokens) and runs attention + FFN for its 512 queries.

Fast path: every GEMM runs as fp8e4 DoubleRow matmuls (2 contraction tiles
per instruction at 0.5 cycles/row). Weights are host-scaled by 64 into the
fp8 normal range; every psum consumer applies the inverse power-of-two
scale for free inside the op it already needed. Softmax exp is split
between the ACT engine (Exp) and gpsimd (pow with base e^(1/128)); all
other ACT work is folded away (biases into host-precomputed vectors, LN
sqrt via gpsimd pow). Post-attention arithmetic is bf16 end to end.
"""

import numpy as np
import ml_dtypes

D = 512
S = 2048
B = 2
H = 8
HD = 64
F = 2048
EPS = 1e-5
NCORES = 8
SQ = 512          # queries per core
P = 128           # partitions
KD = D // P       # 4   D-tiles
KT = S // P       # 16  key tiles
TB = S // 512     # 4   512-token blocks
FT = F // P       # 16  FFN hidden tiles
HP = H // 2       # 4   head pairs
K2 = KT // 2      # 8   key-tile pairs
VC = 96           # padded AV columns: 64 values + 1 ones + 31 zeros

WS = 64.0         # host weight scale
SCALE_QKV = 1.0 / 16.0    # psum *  -> activations stored x4
SCALE_WO = 2.0 ** -12
SCALE_FF2 = 2.0 ** -8
ONES_COL = 1.0 / 16.0     # vh ones column value -> avt = 64*av
EXP_SCALE = 1.0 / 128.0   # scores psum = 16 * true score; softmax /8

BF16 = ml_dtypes.bfloat16
F8 = ml_dtypes.float8_e4m3

# k2 indices whose exp runs on gpsimd (via DVE psum->sbuf copy), per side
POOL_K2 = ({3, 6}, {1, 5})
# FFN1 consume engines per ft tile: 'a' = ACT relu, 'd' = DVE
# (gpsimd cannot read PSUM, so no Pool here)
FT_ENG = "aaaaaaaaaadddddd"

_CACHE = {}
LAST_RESULT = None


def _build_nc():
    import concourse.bacc as bacc
    import concourse.tile as tile
    from concourse import mybir

    bf = mybir.dt.bfloat16
    f8 = mybir.dt.float8e4
    f32 = mybir.dt.float32
    ACT = mybir.ActivationFunctionType
    ALU = mybir.AluOpType
    DR = mybir.MatmulPerfMode.DoubleRow

    nc = bacc.Bacc("TRN2", target_bir_lowering=False, debug=False)

    def din(name, shape, dt=f8):
        return nc.dram_tensor(name, shape, dt, kind="ExternalInput").ap()

    t_q8 = din("q8", [P, KD, SQ])
    t_qbf = din("qbf16", [P, KD, SQ], bf)
    t_kt = din("kt8", [P, KD, S])
    t_vt = din("vt8", [P, KD, S])
    t_wq = din("wq8", [P, KD, D])
    t_wk = din("wk8", [P, KD, D])
    t_wv = din("wv8", [P, KD, D])
    t_wo = din("wo8", [HD, HP, 2, KD, P])
    t_w1 = din("w18", [P, KD, F])
    t_w2 = din("w28", [P, FT, D])
    t_zed = din("zed8", [P, 4096])
    t_bq = din("bq4", [P, KD], f32)
    t_bk = din("bk4", [P, KD], f32)
    t_b1m = din("b1m64", [P, FT], f32)
    t_b1p = din("b1p64", [P, FT], f32)
    t_g1 = din("g1", [P, KD], f32)
    t_be1 = din("be1", [P, KD], f32)
    t_be1p = din("be1p", [P, KD], f32)
    t_g2 = din("g2", [P, KD], f32)
    t_be2 = din("be2", [P, KD], f32)
    t_out = nc.dram_tensor("outT", [P, KD, SQ], bf, kind="ExternalOutput").ap()

    with tile.TileContext(nc) as tc, \
         tc.tile_pool(name="statics", bufs=1) as SP:
        def st(shape, dt, name):
            return SP.tile(shape, dt, tag=name, name=name)

        ones_bf = st([P, 1], bf, "ones_bf")
        nc.gpsimd.memset(ones_bf, 1.0 / D)
        warm_rhs = st([1, SQ], bf, "warm_rhs")
        nc.gpsimd.memset(warm_rhs, 0.0)
        ones_row = st([P, HD], bf, "ones_row")
        nc.gpsimd.memset(ones_row, 1.0)
        ebase = st([P, 2, SQ], bf, "ebase")
        nc.gpsimd.memset(ebase, float(np.exp(EXP_SCALE)))
        nhalf_t = st([1, SQ], f32, "nhalf_t")
        nc.gpsimd.memset(nhalf_t, -0.5)
        eps_w = st([1, 1], bf, "eps_w")
        nc.gpsimd.memset(eps_w, EPS)
        ones_rhs = st([1, SQ], bf, "ones_rhs")
        nc.gpsimd.memset(ones_rhs, 1.0)
        sink0 = st([1, 1], f32, "sink0")
        sink1 = st([1, 1], f32, "sink1")

        # ---- DMAs in first-use order ----
        bq = st([P, KD], f32, "bq")
        nc.sync.dma_start(out=bq, in_=t_bq)
        q8 = st([P, KD, SQ], f8, "q8")
        nc.sync.dma_start(out=q8, in_=t_q8)
        wq = st([P, KD, D], f8, "wq")
        nc.sync.dma_start(out=wq, in_=t_wq)
        wk = st([P, KD, D], f8, "wk")
        nc.sync.dma_start(out=wk, in_=t_wk)
        bk = st([P, KD], f32, "bk")
        nc.sync.dma_start(out=bk, in_=t_bk)
        kt_sb = st([P, KD, S + P], f8, "kt_sb")     # K proj moving data
        nc.sync.dma_start(out=kt_sb[:, :, 0:S // 2], in_=t_kt[:, :, 0:S // 2])

        # persistent activations
        q_z = st([P, KD, 2, SQ], f8, "q_z")         # slot1 = zeros
        kh = st([P, KD, S + P], f8, "kh")           # +128 zero pad for DR dup
        vh = st([P, KT, H, VC], f8, "vh")
        avt = st([P, H, SQ], f8, "avt")
        xres = st([P, KD, SQ], bf, "xres")          # residual; reused as r2
        x1b = st([P, KD, SQ], f8, "x1b")
        x1f = st([P, KD, SQ], bf, "x1f")
        hsb = st([P, FT, SQ], f8, "hsb")
        outsb = st([P, KD, SQ], bf, "outsb")

        nc.sync.dma_start(out=q_z[:, :, 1, :], in_=t_zed[:, 0:KD * SQ].rearrange(
            "p (k s) -> p k s", k=KD))
        nc.sync.dma_start(out=kh[:, :, S:S + P], in_=t_zed[:, 0:KD * P].rearrange(
            "p (k s) -> p k s", k=KD))
        nc.sync.dma_start(out=kt_sb[:, :, S // 2:S], in_=t_kt[:, :, S // 2:S])
        wv = st([P, KD, D], f8, "wv")
        nc.sync.dma_start(out=wv, in_=t_wv)
        vt_sb = st([P, KD, S], f8, "vt_sb")
        nc.sync.dma_start(out=vt_sb[:, :, 0:S // 2], in_=t_vt[:, :, 0:S // 2])
        nc.sync.dma_start(out=vh[:, :, :, HD + 1:VC], in_=t_zed[:, 0:KT * H * 31].rearrange(
            "p (t h c) -> p t h c", t=KT, h=H))
        nc.sync.dma_start(out=vt_sb[:, :, S // 2:S], in_=t_vt[:, :, S // 2:S])
        # tail-phase inputs, queued last
        qbf16 = st([P, KD, SQ], bf, "qbf16")
        nc.sync.dma_start(out=qbf16, in_=t_qbf)
        wo = st([HD, HP, 2, KD, P], f8, "wo")
        nc.sync.dma_start(out=wo, in_=t_wo)
        w1 = st([P, KD, F], f8, "w1")
        nc.sync.dma_start(out=w1, in_=t_w1)
        w2 = st([P, FT, D], f8, "w2")
        nc.sync.dma_start(out=w2, in_=t_w2)
        b1m = st([P, FT], f32, "b1m")
        nc.sync.dma_start(out=b1m, in_=t_b1m)
        b1p = st([P, FT], f32, "b1p")
        nc.sync.dma_start(out=b1p, in_=t_b1p)
        g1 = st([P, KD], f32, "g1")
        nc.sync.dma_start(out=g1, in_=t_g1)
        be1 = st([P, KD], f32, "be1")
        nc.sync.dma_start(out=be1, in_=t_be1)
        be1p = st([P, KD], f32, "be1p")
        nc.sync.dma_start(out=be1p, in_=t_be1p)
        g2 = st([P, KD], f32, "g2")
        nc.sync.dma_start(out=g2, in_=t_g2)
        be2 = st([P, KD], f32, "be2")
        nc.sync.dma_start(out=be2, in_=t_be2)
        SP.seal()

        nc.gpsimd.memset(vh[:, :, :, HD:HD + 1], ONES_COL)

        # ============ phases 1+2: projections interleaved with attention
        with tc.tile_pool(name="att_sb", bufs=1) as asb, \
             tc.tile_pool(name="pj", bufs=2, space="PSUM") as pj, \
             tc.tile_pool(name="sc", bufs=1, space="PSUM") as scp, \
             tc.tile_pool(name="av", bufs=1, space="PSUM") as avp:

            def qgroup(dt):
                ps = pj.tile([P, SQ], f32, tag="pj", name=f"psq{dt}")
                for k in (0, 2):
                    nc.tensor.matmul(ps, wq[:, k:k + 2, dt * P:(dt + 1) * P],
                                     q8[:, k:k + 2, :], start=(k == 0),
                                     stop=(k == 2), perf_mode=DR)
                nc.vector.tensor_scalar(out=q_z[:, dt, 0, :], in0=ps,
                                        scalar1=SCALE_QKV, scalar2=bq[:, dt:dt + 1],
                                        op0=ALU.mult, op1=ALU.add)

            def kgroup(dt, tb):
                def go():
                    tbs = slice(tb * 512, (tb + 1) * 512)
                    ps = pj.tile([P, 512], f32, tag="pj", name=f"psk{dt}_{tb}")
                    for k in (0, 2):
                        nc.tensor.matmul(ps, wk[:, k:k + 2, dt * P:(dt + 1) * P],
                                         kt_sb[:, k:k + 2, tbs], start=(k == 0),
                                         stop=(k == 2), perf_mode=DR)
                    nc.vector.tensor_scalar(out=kh[:, dt, tbs], in0=ps,
                                            scalar1=SCALE_QKV,
                                            scalar2=bk[:, dt:dt + 1],
                                            op0=ALU.mult, op1=ALU.add)
                return go

            def qk_groups(dt):
                return [lambda: qgroup(dt)] + [kgroup(dt, tb) for tb in range(TB)]

            def v_proj(tt):
                ps = pj.tile([P, D], f32, tag="pj", name=f"psv{tt}")
                for k in (0, 2):
                    nc.tensor.matmul(ps, vt_sb[:, k:k + 2, tt * P:(tt + 1) * P],
                                     wv[:, k:k + 2, :], start=(k == 0),
                                     stop=(k == 2), perf_mode=DR)
                nc.vector.tensor_scalar_mul(
                    vh[:, tt, :, 0:HD], ps.rearrange("p (h d) -> p h d", h=H),
                    SCALE_QKV)

            # Wo chains: po psum tiles live in the pj pool (projections are
            # done by the time these run).
            po_tiles = {}

            def wo_mm(dt, hp_i):
                def go():
                    if dt not in po_tiles:
                        po_tiles[dt] = pj.tile([P, SQ], f32, tag="pj",
                                               name=f"po{dt}")
                    nc.tensor.matmul(po_tiles[dt], wo[:, hp_i, :, dt, :],
                                     avt[0:HD, 2 * hp_i:2 * hp_i + 2, :],
                                     start=(hp_i == 0), stop=(hp_i == HP - 1),
                                     perf_mode=DR)
                return go

            def wo_consume(dt):
                po = po_tiles.pop(dt)
                nc.vector.scalar_tensor_tensor(
                    out=xres[:, dt, :], in0=po, scalar=SCALE_WO,
                    in1=qbf16[:, dt, :], op0=ALU.mult, op1=ALU.add)

            # keep-warm matmuls to cover the initial DMA wait
            warm_ps = pj.tile([P, SQ], f32, tag="pj", name="warm_ps")
            for w in range(14):
                nc.tensor.matmul(warm_ps[0:1, :], ones_bf[0:1, 0:1], warm_rhs,
                                 start=(w == 0), stop=(w == 13))
            nc.vector.tensor_scalar(out=sink0, in0=warm_ps[0:1, 0:1],
                                    scalar1=0.0, scalar2=0.0,
                                    op0=ALU.mult, op1=ALU.add)

            qgroup(0)
            for g in [kgroup(0, tb) for tb in range(TB)]:
                g()
            for g in qk_groups(1):
                g()
            fillers = []

            def normalize(hp_n, pav0_n, pav1_n):
                for side, pav in ((0, pav0_n), (1, pav1_n)):
                    h = 2 * hp_n + side
                    rec = asb.tile([1, SQ], bf, tag="rec", bufs=2,
                                   name=f"rec{h}")
                    with nc.allow_low_precision(reason="softmax denom, bf16 ok"):
                        nc.vector.reciprocal(rec, pav[HD:HD + 1, :])
                    rbc = asb.tile([HD, SQ], bf, tag="rbc", bufs=2,
                                   name=f"rbc{h}")
                    nc.gpsimd.partition_broadcast(rbc, rec)
                    nc.vector.tensor_mul(avt[0:HD, h, :], pav[0:HD, :], rbc)

            prev_pavs = None
            for hp in range(HP):
                if hp == 1:
                    fillers += qk_groups(2)
                elif hp == 2:
                    fillers += qk_groups(3)
                elif hp == 3:
                    fillers += [wo_mm(dt, i) for i in range(3) for dt in (0, 1)]
                pav0 = avp.tile([P, SQ], f32, tag="av0")
                pav1 = avp.tile([P, SQ], f32, tag="av1")
                prev = None
                for k2 in range(K2):
                    pscs = []
                    for side in range(2):
                        psc = scp.tile([P, 2, SQ], f32, tag=f"sc{side}", bufs=1)
                        pr = slice(side * HD, side * HD + HD)
                        for i in range(2):
                            kt = 2 * k2 + i
                            lw = kh[pr, hp, kt * P:(kt + 2) * P].rearrange(
                                "p (two k) -> p two k", two=2)
                            nc.tensor.matmul(psc[:, i, :], lw,
                                             q_z[pr, hp, :, :], start=True,
                                             stop=True, perf_mode=DR)
                        pscs.append(psc)
                    ps_ = []
                    for side in range(2):
                        p = asb.tile([P, 2, SQ], f8, tag=f"p{side}", bufs=2)
                        if k2 in POOL_K2[side]:
                            scb = asb.tile([P, 2, SQ], bf, tag=f"scb{side}",
                                           bufs=2)
                            nc.vector.tensor_copy(scb, pscs[side])
                            nc.gpsimd.tensor_tensor(p, ebase, scb, ALU.pow)
                        else:
                            nc.scalar.activation(out=p, in_=pscs[side],
                                                 func=ACT.Exp, scale=EXP_SCALE)
                        ps_.append(p)
                    if k2 == 0 and prev_pavs is not None:
                        normalize(*prev_pavs)
                        prev_pavs = None
                    if hp == 0:
                        v_proj(2 * k2)
                        v_proj(2 * k2 + 1)
                    elif fillers:
                        fillers.pop(0)()
                    if prev is not None:
                        pp, pk2 = prev
                        for side, pav in ((0, pav0), (1, pav1)):
                            nc.tensor.matmul(
                                pav[0:VC, :],
                                vh[:, 2 * pk2:2 * pk2 + 2, 2 * hp + side, :],
                                pp[side], start=(pk2 == 0),
                                stop=False, perf_mode=DR)
                    prev = (ps_, k2)
                pp, pk2 = prev
                for side, pav in ((0, pav0), (1, pav1)):
                    nc.tensor.matmul(
                        pav[0:VC, :], vh[:, 2 * pk2:2 * pk2 + 2, 2 * hp + side, :],
                        pp[side], start=False, stop=True, perf_mode=DR)
                while hp == 3 and fillers:
                    fillers.pop(0)()
                prev_pavs = (hp, pav0, pav1)

            normalize(*prev_pavs)

            # finish Wo for dt 0/1 (head pair 3) + residual
            for dt in (0, 1):
                wo_mm(dt, 3)()
                wo_consume(dt)

        # ============ phase 3: Wo tail + LN1 ============
        def layer_norm(src, gain, outs, stp, tmp, wmp, nwarm):
            """outs: list of (dst, beta_ap, dtype-handled-by-dst)."""
            ps1 = stp.tile([1, SQ], f32, tag="s1")
            ps2 = stp.tile([1, SQ], f32, tag="s2")
            # seed ps2 with eps so var+eps falls out of the sum chain
            nc.tensor.matmul(ps2, eps_w, ones_rhs, start=True, stop=False)
            for dt in range(KD):
                sq = tmp.tile([P, SQ], bf, tag="sq", bufs=2)
                nc.vector.tensor_mul(sq, src[:, dt, :], src[:, dt, :])
                nc.tensor.matmul(ps1, ones_bf, src[:, dt, :], start=(dt == 0),
                                 stop=(dt == KD - 1))
                nc.tensor.matmul(ps2, ones_bf, sq, start=False,
                                 stop=(dt == KD - 1))
            if nwarm:
                warm = wmp.tile([P, SQ], f32, tag="wm", name="lnwarm")
                for w in range(nwarm):
                    nc.tensor.matmul(warm[0:1, :], ones_bf[0:1, 0:1], warm_rhs,
                                     start=(w == 0), stop=(w == nwarm - 1))
                nc.vector.tensor_scalar(out=sink1, in0=warm[0:1, 0:1],
                                        scalar1=0.0, scalar2=0.0,
                                        op0=ALU.mult, op1=ALU.add)
            mean = tmp.tile([1, SQ], f32, tag="ln_mean")
            nc.vector.tensor_copy(mean, ps1)
            m2 = tmp.tile([1, SQ], f32, tag="ln_m2")
            nc.vector.tensor_mul(m2, mean, mean)
            var = tmp.tile([1, SQ], f32, tag="ln_var")
            nc.vector.scalar_tensor_tensor(out=var, in0=m2, scalar=-1.0,
                                           in1=ps2, op0=ALU.mult, op1=ALU.add)
            rstd = tmp.tile([1, SQ], bf, tag="ln_rstd")
            with nc.allow_low_precision(reason="LN rstd, bf16 ok"):
                nc.gpsimd.tensor_tensor(rstd, var, nhalf_t, ALU.pow)
            cvec = tmp.tile([1, SQ], bf, tag="ln_c")
            nc.vector.tensor_mul(cvec, mean, rstd)
            pA = tmp.tile([P, SQ], bf, tag="bA")
            nc.gpsimd.partition_broadcast(pA, rstd)
            pC = tmp.tile([P, SQ], bf, tag="bC")
            nc.gpsimd.partition_broadcast(pC, cvec)
            for dt in range(KD):
                t1 = tmp.tile([P, SQ], bf, tag="t1", bufs=2)
                nc.vector.tensor_mul(t1, src[:, dt, :], pA)
                nc.vector.tensor_sub(t1, t1, pC)
                for dst, beta in outs:
                    nc.vector.tensor_scalar(out=dst[:, dt, :], in0=t1,
                                            scalar1=gain[:, dt:dt + 1],
                                            scalar2=beta[:, dt:dt + 1],
                                            op0=ALU.mult, op1=ALU.add)

        with tc.tile_pool(name="ln1_sb", bufs=1) as tmp1, \
             tc.tile_pool(name="po2", bufs=2, space="PSUM") as pop, \
             tc.tile_pool(name="st1", bufs=1, space="PSUM") as stp1, \
             tc.tile_pool(name="wm1", bufs=1, space="PSUM") as wmp1:
            for dt in (2, 3):
                po_tiles[dt] = pop.tile([P, SQ], f32, tag="po", name=f"po{dt}")
                for hp_i in range(HP):
                    nc.tensor.matmul(po_tiles[dt], wo[:, hp_i, :, dt, :],
                                     avt[0:HD, 2 * hp_i:2 * hp_i + 2, :],
                                     start=(hp_i == 0), stop=(hp_i == HP - 1),
                                     perf_mode=DR)
                wo_consume(dt)
            layer_norm(xres, g1, [(x1b, be1), (x1f, be1p)], stp1, tmp1, wmp1, 14)

        # ============ phase 4: FFN ============
        with tc.tile_pool(name="pf", bufs=5, space="PSUM") as pfp, \
             tc.tile_pool(name="py", bufs=2, space="PSUM") as pyp, \
             tc.tile_pool(name="wm2", bufs=1, space="PSUM") as wmp2:
            for ft in range(FT):
                pf = pfp.tile([P, SQ], f32, tag="pf")
                for k in (0, 2):
                    nc.tensor.matmul(pf, w1[:, k:k + 2, ft * P:(ft + 1) * P],
                                     x1b[:, k:k + 2, :], start=(k == 0),
                                     stop=(k == 2), perf_mode=DR)
                eng = FT_ENG[ft]
                if eng == "a":
                    nc.scalar.activation(out=hsb[:, ft, :], in_=pf, func=ACT.Relu,
                                         bias=b1p[:, ft:ft + 1])
                else:
                    e = nc.vector if eng == "d" else nc.gpsimd
                    e.tensor_scalar(out=hsb[:, ft, :], in0=pf,
                                    scalar1=b1m[:, ft:ft + 1], scalar2=SCALE_QKV,
                                    op0=ALU.max, op1=ALU.mult)
            for dt in range(KD):
                py = pyp.tile([P, SQ], f32, tag="py")
                for f in range(0, FT, 2):
                    nc.tensor.matmul(py, w2[:, f:f + 2, dt * P:(dt + 1) * P],
                                     hsb[:, f:f + 2, :], start=(f == 0),
                                     stop=(f == FT - 2), perf_mode=DR)
                nc.vector.scalar_tensor_tensor(
                    out=xres[:, dt, :], in0=py, scalar=SCALE_FF2,
                    in1=x1f[:, dt, :], op0=ALU.mult, op1=ALU.add)

        # ============ phase 5: LN2 + output ============
        with tc.tile_pool(name="ln2_sb", bufs=1) as tmp2, \
             tc.tile_pool(name="st2", bufs=1, space="PSUM") as stp2, \
             tc.tile_pool(name="wm3", bufs=1, space="PSUM") as wmp3:
            layer_norm(xres, g2, [(outsb, be2)], stp2, tmp2, wmp3, 0)
            for dt in range(KD):
                nc.sync.dma_start(out=t_out[:, dt, :], in_=outsb[:, dt, :])

    nc.compile()
    return nc


def _get_nc():
    if "nc" not in _CACHE:
        _CACHE["nc"] = _build_nc()
    return _CACHE["nc"]


def make_in_maps(q, k, v, Wq, bq, Wk, bk, Wv, bv, Wo, bo, W1, b1, W2, b2,
                 g1, be1, g2, be2):
    f32 = np.float32
    q = np.asarray(q, f32)
    k = np.asarray(k, f32)
    v = np.asarray(v, f32)
    Wq, Wk, Wv, Wo = (np.asarray(x, f32) for x in (Wq, Wk, Wv, Wo))
    W1, W2 = np.asarray(W1, f32), np.asarray(W2, f32)
    bq, bk, bv, bo = (np.asarray(x, f32) for x in (bq, bk, bv, bo))
    b1, b2 = np.asarray(b1, f32), np.asarray(b2, f32)
    g1, be1, g2, be2 = (np.asarray(x, f32) for x in (g1, be1, g2, be2))

    def tile_pd(x, n):  # [n*P] -> [P, n]
        return np.ascontiguousarray(x.reshape(n, P).T)

    def wt8(w, cols):  # [out, in] -> [P, in//P, cols] scaled fp8
        return np.ascontiguousarray(
            (w.T * WS).reshape(-1, P, cols).transpose(1, 0, 2)).astype(F8)

    # per-row (hidden-unit) scale for W2: ACT-relu rows hold 64h -> x4,
    # max-trick rows hold 4t -> x64
    row_scale = np.empty(F, f32)
    t_mask = np.zeros(F, bool)
    for ft in range(FT):
        sl = slice(ft * P, (ft + 1) * P)
        if FT_ENG[ft] == "a":
            row_scale[sl] = 4.0
        else:
            row_scale[sl] = 64.0
            t_mask[sl] = True
    w28 = np.ascontiguousarray(
        (W2.T * row_scale[:, None]).reshape(FT, P, D).transpose(1, 0, 2)).astype(F8)

    bo_eff = bo + Wo @ bv
    b2_eff = b2 + W2[:, t_mask] @ b1[t_mask]

    wo8 = np.ascontiguousarray(
        (Wo.T * WS).reshape(HP, 2, HD, KD, P).transpose(2, 0, 1, 3, 4)).astype(F8)

    shared = {
        "wq8": wt8(Wq, D), "wk8": wt8(Wk, D), "wv8": wt8(Wv, D),
        "w18": wt8(W1, F), "w28": w28, "wo8": wo8,
        "zed8": np.zeros((P, 4096), F8),
        "bq4": tile_pd(4.0 * bq, KD), "bk4": tile_pd(4.0 * bk, KD),
        "b1m64": tile_pd(-64.0 * b1, FT), "b1p64": tile_pd(64.0 * b1, FT),
        "g1": tile_pd(g1, KD), "be1": tile_pd(be1, KD),
        "be1p": tile_pd(be1 + b2_eff, KD),
        "g2": tile_pd(g2, KD), "be2": tile_pd(be2, KD),
    }

    def fm(x, dt):  # [S, D] -> [P, KD, S] feature-major
        return np.ascontiguousarray(
            x.T.reshape(KD, P, -1).transpose(1, 0, 2)).astype(dt)

    kts = [fm(k[b], F8) for b in range(B)]
    vts = [fm(v[b], F8) for b in range(B)]

    in_maps = []
    for c in range(NCORES):
        b, s0 = c // 4, (c % 4) * SQ
        qs = q[b, s0:s0 + SQ, :]
        in_maps.append({
            "q8": fm(qs, F8),
            "qbf16": fm(qs + bo_eff, BF16),
            "kt8": kts[b], "vt8": vts[b], **shared,
        })
    return in_maps


def assemble_out(results):
    out = np.empty((B, S, D), np.float32)
    for c in range(NCORES):
        b, s0 = c // 4, (c % 4) * SQ
        out[b, s0:s0 + SQ, :] = results[c]["outT"].astype(np.float32) \
            .transpose(2, 1, 0).reshape(SQ, D)
    return out


def kernel(**inputs):
    global LAST_RESULT
    import os

    from concourse.bass_utils import run_bass_kernel_spmd

    nc = _get_nc()
    in_maps = make_in_maps(**inputs)
    try:
        res = run_bass_kernel_spmd(nc, in_maps, core_ids=list(range(NCORES)))
    except ModuleNotFoundError:
        # BASS_TRACE set but this container has no axon NTFF profile hook
        # (antenv.axon_hooks missing) — rerun untraced.
        os.environ["BASS_NEVER_TRACE"] = "1"
        res = run_bass_kernel_spmd(nc, in_maps, core_ids=list(range(NCORES)))
    LAST_RESULT = res
    return assemble_out(res.results)


# revision 20
# speedup vs baseline: 1.4279x; 1.0115x over previous
"""Trainium2 Bass kernel for a post-norm transformer encoder layer.

Contract: kernel(**inputs) takes the FULL fp32 inputs (as produced by the
problem's setup_inputs) and returns the FULL [2, 2048, 512] fp32 output.

Sharding (8 cores, no collectives): core c owns 512 query tokens of batch
c // 4 (slice (c % 4) * 512). Each core recomputes the K/V projections for
its whole batch (2048 tokens) and runs attention + FFN for its 512 queries.

Fast path: every GEMM runs as fp8e4 DoubleRow matmuls (2 contraction tiles
per instruction at 0.5 cycles/row). Weights are host-scaled by 64 into the
fp8 normal range; every psum consumer applies the inverse power-of-two
scale for free inside the op it already needed. Softmax exp is split
between the ACT engine (Exp) and gpsimd (pow with base e^(1/128)); all
other ACT work is folded away (biases into host-precomputed vectors, LN
sqrt via gpsimd pow). Post-attention arithmetic is bf16 end to end.
"""

import numpy as np
import ml_dtypes

D = 512
S = 2048
B = 2
H = 8
HD = 64
F = 2048
EPS = 1e-5
NCORES = 8
SQ = 512          # queries per core
P = 128           # partitions
KD = D // P       # 4   D-tiles
KT = S // P       # 16  key tiles
TB = S // 512     # 4   512-token blocks
FT = F // P       # 16  FFN hidden tiles
HP = H // 2       # 4   head pairs
K2 = KT // 2      # 8   key-tile pairs
VC = 96           # padded AV columns: 64 values + 1 ones + 31 zeros

WS = 64.0         # host weight scale
SCALE_QKV = 1.0 / 16.0    # psum *  -> activations stored x4
SCALE_WO = 2.0 ** -12
SCALE_FF2 = 2.0 ** -8
ONES_COL = 1.0 / 16.0     # vh ones column value -> avt = 64*av
EXP_SCALE = 1.0 / 128.0   # scores psum = 16 * true score; softmax /8

BF16 = ml_dtypes.bfloat16
F8 = ml_dtypes.float8_e4m3

# k2 indices whose exp runs on gpsimd (via DVE psum->sbuf copy), per side
POOL_K2 = ({3, 6}, {1, 5})
# FFN1 consume engines per ft tile: 'a' = ACT relu, 'd' = DVE
# (gpsimd cannot read PSUM, so no Pool here)
FT_ENG = "aaaaaaaaaadddddd"

_CACHE = {}
LAST_RESULT = None


def _build_nc():
    import concourse.bacc as bacc
    import concourse.tile as tile
    from concourse import mybir

    bf = mybir.dt.bfloat16
    f8 = mybir.dt.float8e4
    f32 = mybir.dt.float32
    ACT = mybir.ActivationFunctionType
    ALU = mybir.AluOpType
    DR = mybir.MatmulPerfMode.DoubleRow

    nc = bacc.Bacc("TRN2", target_bir_lowering=False, debug=False)

    def din(name, shape, dt=f8):
        return nc.dram_tensor(name, shape, dt, kind="ExternalInput").ap()

    t_q8 = din("q8", [P, KD, SQ])
    t_qbf = din("qbf16", [P, KD, SQ], bf)
    t_kt = din("kt8", [P, KD, S])
    t_vt = din("vt8", [P, KD, S])
    t_wq = din("wq8", [P, KD, D])
    t_wk = din("wk8", [P, KD, D])
    t_wv = din("wv8", [P, KD, D])
    t_wo = din("wo8", [HD, HP, 2, KD, P])
    t_w1 = din("w18", [P, KD, F])
    t_w2 = din("w28", [P, FT, D])
    t_bq = din("bq4", [P, KD], f32)
    t_bk = din("bk4", [P, KD], f32)
    t_b1m = din("b1m64", [P, FT], f32)
    t_b1p = din("b1p64", [P, FT], f32)
    t_g1 = din("g1", [P, KD], f32)
    t_be1 = din("be1", [P, KD], f32)
    t_be1p = din("be1p", [P, KD], f32)
    t_g2 = din("g2", [P, KD], f32)
    t_be2 = din("be2", [P, KD], f32)
    t_out = nc.dram_tensor("outT", [P, KD, SQ], bf, kind="ExternalOutput").ap()

    with tile.TileContext(nc) as tc, \
         tc.tile_pool(name="statics", bufs=1) as SP:
        def st(shape, dt, name):
            return SP.tile(shape, dt, tag=name, name=name)

        ones_bf = st([P, 1], bf, "ones_bf")
        nc.gpsimd.memset(ones_bf, 1.0 / D)
        warm_rhs = st([1, SQ], bf, "warm_rhs")
        nc.gpsimd.memset(warm_rhs, 0.0)
        ones_row = st([P, HD], bf, "ones_row")
        nc.gpsimd.memset(ones_row, 1.0)
        ebase = st([P, 2, SQ], bf, "ebase")
        nc.gpsimd.memset(ebase, float(np.exp(EXP_SCALE)))
        nhalf_t = st([1, SQ], f32, "nhalf_t")
        nc.gpsimd.memset(nhalf_t, -0.5)
        eps_w = st([1, 1], bf, "eps_w")
        nc.gpsimd.memset(eps_w, EPS)
        ones_rhs = st([1, SQ], bf, "ones_rhs")
        nc.gpsimd.memset(ones_rhs, 1.0)
        sink0 = st([1, 1], f32, "sink0")
        sink1 = st([1, 1], f32, "sink1")

        # persistent activations
        q_z = st([P, KD, 2, SQ], f8, "q_z")         # slot1 = zeros
        kh = st([P, KD, S + P], f8, "kh")           # +128 zero pad for DR dup
        vh = st([P, KT, H, VC], f8, "vh")
        avt = st([P, H, SQ], f8, "avt")
        xres = st([P, KD, SQ], bf, "xres")          # residual; reused as r2
        x1b = st([P, KD, SQ], f8, "x1b")
        x1f = st([P, KD, SQ], bf, "x1f")
        hsb = st([P, FT, SQ], f8, "hsb")
        outsb = st([P, KD, SQ], bf, "outsb")

        # zero regions via gpsimd (Pool is idle until the first pool-exp)
        nc.gpsimd.memset(q_z[:, :, 1, :], 0.0)
        nc.gpsimd.memset(kh[:, :, S:S + P], 0.0)
        nc.gpsimd.memset(vh[:, :, :, HD + 1:VC], 0.0)

        # ---- DMAs in first-use order; kt/vt interleaved per 512-token block
        bq = st([P, KD], f32, "bq")
        nc.sync.dma_start(out=bq, in_=t_bq)
        q8 = st([P, KD, SQ], f8, "q8")
        nc.sync.dma_start(out=q8, in_=t_q8)
        wq = st([P, KD, D], f8, "wq")
        nc.sync.dma_start(out=wq, in_=t_wq)
        wk = st([P, KD, D], f8, "wk")
        nc.sync.dma_start(out=wk, in_=t_wk)
        bk = st([P, KD], f32, "bk")
        nc.sync.dma_start(out=bk, in_=t_bk)
        wv = st([P, KD, D], f8, "wv")
        nc.sync.dma_start(out=wv, in_=t_wv)
        kt_sb = st([P, KD, S + P], f8, "kt_sb")     # K proj moving data
        vt_sb = st([P, KD, S], f8, "vt_sb")
        for tb in range(TB):
            tbs = slice(tb * 512, (tb + 1) * 512)
            nc.sync.dma_start(out=kt_sb[:, :, tbs], in_=t_kt[:, :, tbs])
            nc.sync.dma_start(out=vt_sb[:, :, tbs], in_=t_vt[:, :, tbs])
        # tail-phase inputs, queued last
        qbf16 = st([P, KD, SQ], bf, "qbf16")
        nc.sync.dma_start(out=qbf16, in_=t_qbf)
        wo = st([HD, HP, 2, KD, P], f8, "wo")
        nc.sync.dma_start(out=wo, in_=t_wo)
        w1 = st([P, KD, F], f8, "w1")
        nc.sync.dma_start(out=w1, in_=t_w1)
        w2 = st([P, FT, D], f8, "w2")
        nc.sync.dma_start(out=w2, in_=t_w2)
        b1m = st([P, FT], f32, "b1m")
        nc.sync.dma_start(out=b1m, in_=t_b1m)
        b1p = st([P, FT], f32, "b1p")
        nc.sync.dma_start(out=b1p, in_=t_b1p)
        g1 = st([P, KD], f32, "g1")
        nc.sync.dma_start(out=g1, in_=t_g1)
        be1 = st([P, KD], f32, "be1")
        nc.sync.dma_start(out=be1, in_=t_be1)
        be1p = st([P, KD], f32, "be1p")
        nc.sync.dma_start(out=be1p, in_=t_be1p)
        g2 = st([P, KD], f32, "g2")
        nc.sync.dma_start(out=g2, in_=t_g2)
        be2 = st([P, KD], f32, "be2")
        nc.sync.dma_start(out=be2, in_=t_be2)
        SP.seal()

        nc.gpsimd.memset(vh[:, :, :, HD:HD + 1], ONES_COL)

        # ============ phases 1+2: projections interleaved with attention
        with tc.tile_pool(name="att_sb", bufs=1) as asb, \
             tc.tile_pool(name="pj", bufs=2, space="PSUM") as pj, \
             tc.tile_pool(name="sc", bufs=1, space="PSUM") as scp, \
             tc.tile_pool(name="av", bufs=1, space="PSUM") as avp:

            def qgroup(dt):
                ps = pj.tile([P, SQ], f32, tag="pj", name=f"psq{dt}")
                for k in (0, 2):
                    nc.tensor.matmul(ps, wq[:, k:k + 2, dt * P:(dt + 1) * P],
                                     q8[:, k:k + 2, :], start=(k == 0),
                                     stop=(k == 2), perf_mode=DR)
                nc.vector.tensor_scalar(out=q_z[:, dt, 0, :], in0=ps,
                                        scalar1=SCALE_QKV, scalar2=bq[:, dt:dt + 1],
                                        op0=ALU.mult, op1=ALU.add)

            def kgroup(dt, tb):
                def go():
                    tbs = slice(tb * 512, (tb + 1) * 512)
                    ps = pj.tile([P, 512], f32, tag="pj", name=f"psk{dt}_{tb}")
                    for k in (0, 2):
                        nc.tensor.matmul(ps, wk[:, k:k + 2, dt * P:(dt + 1) * P],
                                         kt_sb[:, k:k + 2, tbs], start=(k == 0),
                                         stop=(k == 2), perf_mode=DR)
                    nc.vector.tensor_scalar(out=kh[:, dt, tbs], in0=ps,
                                            scalar1=SCALE_QKV,
                                            scalar2=bk[:, dt:dt + 1],
                                            op0=ALU.mult, op1=ALU.add)
                return go

            def qk_groups(dt):
                return [lambda: qgroup(dt)] + [kgroup(dt, tb) for tb in range(TB)]

            def v_proj(tt):
                ps = pj.tile([P, D], f32, tag="pj", name=f"psv{tt}")
                for k in (0, 2):
                    nc.tensor.matmul(ps, vt_sb[:, k:k + 2, tt * P:(tt + 1) * P],
                                     wv[:, k:k + 2, :], start=(k == 0),
                                     stop=(k == 2), perf_mode=DR)
                nc.vector.tensor_scalar_mul(
                    vh[:, tt, :, 0:HD], ps.rearrange("p (h d) -> p h d", h=H),
                    SCALE_QKV)

            # Wo chains: po psum tiles live in the pj pool (projections are
            # done by the time these run).
            po_tiles = {}

            def wo_mm(dt, hp_i):
                def go():
                    if dt not in po_tiles:
                        po_tiles[dt] = pj.tile([P, SQ], f32, tag="pj",
                                               name=f"po{dt}")
                    nc.tensor.matmul(po_tiles[dt], wo[:, hp_i, :, dt, :],
                                     avt[0:HD, 2 * hp_i:2 * hp_i + 2, :],
                                     start=(hp_i == 0), stop=(hp_i == HP - 1),
                                     perf_mode=DR)
                return go

            def wo_consume(dt):
                po = po_tiles.pop(dt)
                nc.vector.scalar_tensor_tensor(
                    out=xres[:, dt, :], in0=po, scalar=SCALE_WO,
                    in1=qbf16[:, dt, :], op0=ALU.mult, op1=ALU.add)

            # keep-warm matmuls to cover the initial DMA wait
            warm_ps = pj.tile([P, SQ], f32, tag="pj", name="warm_ps")
            for w in range(8):
                nc.tensor.matmul(warm_ps[0:1, :], ones_bf[0:1, 0:1], warm_rhs,
                                 start=(w == 0), stop=(w == 7))
            nc.vector.tensor_scalar(out=sink0, in0=warm_ps[0:1, 0:1],
                                    scalar1=0.0, scalar2=0.0,
                                    op0=ALU.mult, op1=ALU.add)

            qgroup(0)
            for g in [kgroup(0, tb) for tb in range(TB)]:
                g()
            for g in qk_groups(1):
                g()
            fillers = []

            def normalize(hp_n, pav0_n, pav1_n):
                for side, pav in ((0, pav0_n), (1, pav1_n)):
                    h = 2 * hp_n + side
                    rec = asb.tile([1, SQ], bf, tag="rec", bufs=2,
                                   name=f"rec{h}")
                    with nc.allow_low_precision(reason="softmax denom, bf16 ok"):
                        nc.vector.reciprocal(rec, pav[HD:HD + 1, :])
                    rbc = asb.tile([HD, SQ], bf, tag="rbc", bufs=2,
                                   name=f"rbc{h}")
                    nc.gpsimd.partition_broadcast(rbc, rec)
                    nc.vector.tensor_mul(avt[0:HD, h, :], pav[0:HD, :], rbc)

            prev_pavs = None
            for hp in range(HP):
                if hp == 1:
                    fillers += qk_groups(2)
                elif hp == 2:
                    fillers += qk_groups(3)
                elif hp == 3:
                    fillers += [wo_mm(dt, i) for i in range(3) for dt in (0, 1)]
                pav0 = avp.tile([P, SQ], f32, tag="av0")
                pav1 = avp.tile([P, SQ], f32, tag="av1")
                prev = None
                for k2 in range(K2):
                    pscs = []
                    for side in range(2):
                        psc = scp.tile([P, 2, SQ], f32, tag=f"sc{side}", bufs=1)
                        pr = slice(side * HD, side * HD + HD)
                        for i in range(2):
                            kt = 2 * k2 + i
                            lw = kh[pr, hp, kt * P:(kt + 2) * P].rearrange(
                                "p (two k) -> p two k", two=2)
                            nc.tensor.matmul(psc[:, i, :], lw,
                                             q_z[pr, hp, :, :], start=True,
                                             stop=True, perf_mode=DR)
                        pscs.append(psc)
                    ps_ = []
                    for side in range(2):
                        p = asb.tile([P, 2, SQ], f8, tag=f"p{side}", bufs=2)
                        if k2 in POOL_K2[side]:
                            scb = asb.tile([P, 2, SQ], bf, tag=f"scb{side}",
                                           bufs=2)
                            nc.vector.tensor_copy(scb, pscs[side])
                            nc.gpsimd.tensor_tensor(p, ebase, scb, ALU.pow)
                        else:
                            nc.scalar.activation(out=p, in_=pscs[side],
                                                 func=ACT.Exp, scale=EXP_SCALE)
                        ps_.append(p)
                    if k2 == 0 and prev_pavs is not None:
                        normalize(*prev_pavs)
                        prev_pavs = None
                    if hp == 0:
                        v_proj(2 * k2)
                        v_proj(2 * k2 + 1)
                    elif fillers:
                        fillers.pop(0)()
                    if prev is not None:
                        pp, pk2 = prev
                        for side, pav in ((0, pav0), (1, pav1)):
                            nc.tensor.matmul(
                                pav[0:VC, :],
                                vh[:, 2 * pk2:2 * pk2 + 2, 2 * hp + side, :],
                                pp[side], start=(pk2 == 0),
                                stop=False, perf_mode=DR)
                    prev = (ps_, k2)
                pp, pk2 = prev
                for side, pav in ((0, pav0), (1, pav1)):
                    nc.tensor.matmul(
                        pav[0:VC, :], vh[:, 2 * pk2:2 * pk2 + 2, 2 * hp + side, :],
                        pp[side], start=False, stop=True, perf_mode=DR)
                while hp == 3 and fillers:
                    fillers.pop(0)()
                prev_pavs = (hp, pav0, pav1)

            normalize(*prev_pavs)

            # finish Wo for dt 0/1 (head pair 3) + residual
            for dt in (0, 1):
                wo_mm(dt, 3)()
                wo_consume(dt)

        # ============ phase 3: Wo tail + LN1 ============
        def layer_norm(src, gain, outs, stp, tmp, wmp, nwarm):
            """outs: list of (dst, beta_ap, dtype-handled-by-dst)."""
            ps1 = stp.tile([1, SQ], f32, tag="s1")
            ps2 = stp.tile([1, SQ], f32, tag="s2")
            # seed ps2 with eps so var+eps falls out of the sum chain
            nc.tensor.matmul(ps2, eps_w, ones_rhs, start=True, stop=False)
            for dt in range(KD):
                sq = tmp.tile([P, SQ], bf, tag="sq", bufs=2)
                nc.vector.tensor_mul(sq, src[:, dt, :], src[:, dt, :])
                nc.tensor.matmul(ps1, ones_bf, src[:, dt, :], start=(dt == 0),
                                 stop=(dt == KD - 1))
                nc.tensor.matmul(ps2, ones_bf, sq, start=False,
                                 stop=(dt == KD - 1))
            if nwarm:
                warm = wmp.tile([P, SQ], f32, tag="wm", name="lnwarm")
                for w in range(nwarm):
                    nc.tensor.matmul(warm[0:1, :], ones_bf[0:1, 0:1], warm_rhs,
                                     start=(w == 0), stop=(w == nwarm - 1))
                nc.vector.tensor_scalar(out=sink1, in0=warm[0:1, 0:1],
                                        scalar1=0.0, scalar2=0.0,
                                        op0=ALU.mult, op1=ALU.add)
            mean = tmp.tile([1, SQ], f32, tag="ln_mean")
            nc.vector.tensor_copy(mean, ps1)
            m2 = tmp.tile([1, SQ], f32, tag="ln_m2")
            nc.vector.tensor_mul(m2, mean, mean)
            var = tmp.tile([1, SQ], f32, tag="ln_var")
            nc.vector.scalar_tensor_tensor(out=var, in0=m2, scalar=-1.0,
                                           in1=ps2, op0=ALU.mult, op1=ALU.add)
            rstd = tmp.tile([1, SQ], bf, tag="ln_rstd")
            with nc.allow_low_precision(reason="LN rstd, bf16 ok"):
                nc.gpsimd.tensor_tensor(rstd, var, nhalf_t, ALU.pow)
            cvec = tmp.tile([1, SQ], bf, tag="ln_c")
            nc.vector.tensor_mul(cvec, mean, rstd)
            pA = tmp.tile([P, SQ], bf, tag="bA")
            nc.gpsimd.partition_broadcast(pA, rstd)
            pC = tmp.tile([P, SQ], bf, tag="bC")
            nc.gpsimd.partition_broadcast(pC, cvec)
            for dt in range(KD):
                t1 = tmp.tile([P, SQ], bf, tag="t1", bufs=2)
                nc.vector.tensor_mul(t1, src[:, dt, :], pA)
                nc.vector.tensor_sub(t1, t1, pC)
                for dst, beta in outs:
                    nc.vector.tensor_scalar(out=dst[:, dt, :], in0=t1,
                                            scalar1=gain[:, dt:dt + 1],
                                            scalar2=beta[:, dt:dt + 1],
                                            op0=ALU.mult, op1=ALU.add)

        with tc.tile_pool(name="ln1_sb", bufs=1) as tmp1, \
             tc.tile_pool(name="po2", bufs=2, space="PSUM") as pop, \
             tc.tile_pool(name="st1", bufs=1, space="PSUM") as stp1, \
             tc.tile_pool(name="wm1", bufs=1, space="PSUM") as wmp1:
            for dt in (2, 3):
                po_tiles[dt] = pop.tile([P, SQ], f32, tag="po", name=f"po{dt}")
                for hp_i in range(HP):
                    nc.tensor.matmul(po_tiles[dt], wo[:, hp_i, :, dt, :],
                                     avt[0:HD, 2 * hp_i:2 * hp_i + 2, :],
                                     start=(hp_i == 0), stop=(hp_i == HP - 1),
                                     perf_mode=DR)
                wo_consume(dt)
            layer_norm(xres, g1, [(x1b, be1), (x1f, be1p)], stp1, tmp1, wmp1, 14)

        # ============ phase 4: FFN ============
        with tc.tile_pool(name="pf", bufs=5, space="PSUM") as pfp, \
             tc.tile_pool(name="py", bufs=2, space="PSUM") as pyp, \
             tc.tile_pool(name="wm2", bufs=1, space="PSUM") as wmp2:
            for ft in range(FT):
                pf = pfp.tile([P, SQ], f32, tag="pf")
                for k in (0, 2):
                    nc.tensor.matmul(pf, w1[:, k:k + 2, ft * P:(ft + 1) * P],
                                     x1b[:, k:k + 2, :], start=(k == 0),
                                     stop=(k == 2), perf_mode=DR)
                eng = FT_ENG[ft]
                if eng == "a":
                    nc.scalar.activation(out=hsb[:, ft, :], in_=pf, func=ACT.Relu,
                                         bias=b1p[:, ft:ft + 1])
                else:
                    e = nc.vector if eng == "d" else nc.gpsimd
                    e.tensor_scalar(out=hsb[:, ft, :], in0=pf,
                                    scalar1=b1m[:, ft:ft + 1], scalar2=SCALE_QKV,
                                    op0=ALU.max, op1=ALU.mult)
            for dt in range(KD):
                py = pyp.tile([P, SQ], f32, tag="py")
                for f in range(0, FT, 2):
                    nc.tensor.matmul(py, w2[:, f:f + 2, dt * P:(dt + 1) * P],
                                     hsb[:, f:f + 2, :], start=(f == 0),
                                     stop=(f == FT - 2), perf_mode=DR)
                nc.vector.scalar_tensor_tensor(
                    out=xres[:, dt, :], in0=py, scalar=SCALE_FF2,
                    in1=x1f[:, dt, :], op0=ALU.mult, op1=ALU.add)

        # ============ phase 5: LN2 + output ============
        with tc.tile_pool(name="ln2_sb", bufs=1) as tmp2, \
             tc.tile_pool(name="st2", bufs=1, space="PSUM") as stp2, \
             tc.tile_pool(name="wm3", bufs=1, space="PSUM") as wmp3:
            layer_norm(xres, g2, [(outsb, be2)], stp2, tmp2, wmp3, 0)
            for dt in range(KD):
                nc.sync.dma_start(out=t_out[:, dt, :], in_=outsb[:, dt, :])

    nc.compile()
    return nc


def _get_nc():
    if "nc" not in _CACHE:
        _CACHE["nc"] = _build_nc()
    return _CACHE["nc"]


def make_in_maps(q, k, v, Wq, bq, Wk, bk, Wv, bv, Wo, bo, W1, b1, W2, b2,
                 g1, be1, g2, be2):
    f32 = np.float32
    q = np.asarray(q, f32)
    k = np.asarray(k, f32)
    v = np.asarray(v, f32)
    Wq, Wk, Wv, Wo = (np.asarray(x, f32) for x in (Wq, Wk, Wv, Wo))
    W1, W2 = np.asarray(W1, f32), np.asarray(W2, f32)
    bq, bk, bv, bo = (np.asarray(x, f32) for x in (bq, bk, bv, bo))
    b1, b2 = np.asarray(b1, f32), np.asarray(b2, f32)
    g1, be1, g2, be2 = (np.asarray(x, f32) for x in (g1, be1, g2, be2))

    def tile_pd(x, n):  # [n*P] -> [P, n]
        return np.ascontiguousarray(x.reshape(n, P).T)

    def wt8(w, cols):  # [out, in] -> [P, in//P, cols] scaled fp8
        return np.ascontiguousarray(
            (w.T * WS).reshape(-1, P, cols).transpose(1, 0, 2)).astype(F8)

    # per-row (hidden-unit) scale for W2: ACT-relu rows hold 64h -> x4,
    # max-trick rows hold 4t -> x64
    row_scale = np.empty(F, f32)
    t_mask = np.zeros(F, bool)
    for ft in range(FT):
        sl = slice(ft * P, (ft + 1) * P)
        if FT_ENG[ft] == "a":
            row_scale[sl] = 4.0
        else:
            row_scale[sl] = 64.0
            t_mask[sl] = True
    w28 = np.ascontiguousarray(
        (W2.T * row_scale[:, None]).reshape(FT, P, D).transpose(1, 0, 2)).astype(F8)

    bo_eff = bo + Wo @ bv
    b2_eff = b2 + W2[:, t_mask] @ b1[t_mask]

    wo8 = np.ascontiguousarray(
        (Wo.T * WS).reshape(HP, 2, HD, KD, P).transpose(2, 0, 1, 3, 4)).astype(F8)

    shared = {
        "wq8": wt8(Wq, D), "wk8": wt8(Wk, D), "wv8": wt8(Wv, D),
        "w18": wt8(W1, F), "w28": w28, "wo8": wo8,
        "bq4": tile_pd(4.0 * bq, KD), "bk4": tile_pd(4.0 * bk, KD),
        "b1m64": tile_pd(-64.0 * b1, FT), "b1p64": tile_pd(64.0 * b1, FT),
        "g1": tile_pd(g1, KD), "be1": tile_pd(be1, KD),
        "be1p": tile_pd(be1 + b2_eff, KD),
        "g2": tile_pd(g2, KD), "be2": tile_pd(be2, KD),
    }

    def fm(x, dt):  # [S, D] -> [P, KD, S] feature-major
        return np.ascontiguousarray(
            x.T.reshape(KD, P, -1).transpose(1, 0, 2)).astype(dt)

    kts = [fm(k[b], F8) for b in range(B)]
    vts = [fm(v[b], F8) for b in range(B)]

    in_maps = []
    for c in range(NCORES):
        b, s0 = c // 4, (c % 4) * SQ
        qs = q[b, s0:s0 + SQ, :]
        in_maps.append({
            "q8": fm(qs, F8),
            "qbf16": fm(qs + bo_eff, BF16),
            "kt8": kts[b], "vt8": vts[b], **shared,
        })
    return in_maps


def assemble_out(results):
    out = np.empty((B, S, D), np.float32)
    for c in range(NCORES):
        b, s0 = c // 4, (c % 4) * SQ
        out[b, s0:s0 + SQ, :] = results[c]["outT"].astype(np.float32) \
            .transpose(2, 1, 0).reshape(SQ, D)
    return out


def kernel(**inputs):
    global LAST_RESULT
    import os

    from concourse.bass_utils import run_bass_kernel_spmd

    nc = _get_nc()
    in_maps = make_in_maps(**inputs)
    try:
        res = run_bass_kernel_spmd(nc, in_maps, core_ids=list(range(NCORES)))
    except ModuleNotFoundError:
        # BASS_TRACE set but this container has no axon NTFF profile hook
        # (antenv.axon_hooks missing) — rerun untraced.
        os.environ["BASS_NEVER_TRACE"] = "1"
        res = run_bass_kernel_spmd(nc, in_maps, core_ids=list(range(NCORES)))
    LAST_RESULT = res
    return assemble_out(res.results)


# revision 22
# speedup vs baseline: 1.4351x; 1.0051x over previous
"""Trainium2 Bass kernel for a post-norm transformer encoder layer.

Contract: kernel(**inputs) takes the FULL fp32 inputs (as produced by the
problem's setup_inputs) and returns the FULL [2, 2048, 512] fp32 output.

Sharding (8 cores, no collectives): core c owns 512 query tokens of batch
c // 4 (slice (c % 4) * 512). Each core recomputes the K/V projections for
its whole batch (2048 tokens) and runs attention + FFN for its 512 queries.

Fast path: every GEMM runs as fp8e4 DoubleRow matmuls (2 contraction tiles
per instruction at 0.5 cycles/row). Weights are host-scaled by 64 into the
fp8 normal range; every psum consumer applies the inverse power-of-two
scale for free inside the op it already needed. Softmax exp is split
between the ACT engine (Exp) and gpsimd (pow with base e^(1/128)); all
other ACT work is folded away (biases into host-precomputed vectors, LN
sqrt via gpsimd pow). Post-attention arithmetic is bf16 end to end.
"""

import numpy as np
import ml_dtypes

D = 512
S = 2048
B = 2
H = 8
HD = 64
F = 2048
EPS = 1e-5
NCORES = 8
SQ = 512          # queries per core
P = 128           # partitions
KD = D // P       # 4   D-tiles
KT = S // P       # 16  key tiles
TB = S // 512     # 4   512-token blocks
FT = F // P       # 16  FFN hidden tiles
HP = H // 2       # 4   head pairs
K2 = KT // 2      # 8   key-tile pairs
VC = 96           # padded AV columns: 64 values + 1 ones + 31 zeros

WS = 64.0         # host weight scale
SCALE_QKV = 1.0 / 16.0    # psum *  -> activations stored x4
SCALE_WO = 2.0 ** -12
SCALE_FF2 = 2.0 ** -8
ONES_COL = 1.0 / 16.0     # vh ones column value -> avt = 64*av
EXP_SCALE = 1.0 / 128.0   # scores psum = 16 * true score; softmax /8

BF16 = ml_dtypes.bfloat16
F8 = ml_dtypes.float8_e4m3

# k2 indices whose exp runs on gpsimd (via DVE psum->sbuf copy), per side
POOL_K2 = ({3, 6}, {1, 5})
# FFN1 consume engines per ft tile: 'a' = ACT relu, 'd' = DVE
# (gpsimd cannot read PSUM, so no Pool here)
FT_ENG = "aaaaaaaaaadddddd"

_CACHE = {}
LAST_RESULT = None


def _build_nc():
    import concourse.bacc as bacc
    import concourse.tile as tile
    from concourse import mybir

    bf = mybir.dt.bfloat16
    f8 = mybir.dt.float8e4
    f32 = mybir.dt.float32
    ACT = mybir.ActivationFunctionType
    ALU = mybir.AluOpType
    DR = mybir.MatmulPerfMode.DoubleRow

    nc = bacc.Bacc("TRN2", target_bir_lowering=False, debug=False)

    def din(name, shape, dt=f8):
        return nc.dram_tensor(name, shape, dt, kind="ExternalInput").ap()

    t_q8 = din("q8", [P, KD, SQ])
    t_qbf = din("qbf16", [P, KD, SQ], bf)
    t_kt = din("kt8", [P, KD, S])
    t_vt = din("vt8", [P, KD, S])
    t_wq = din("wq8", [P, KD, D])
    t_wk = din("wk8", [P, KD, D])
    t_wv = din("wv8", [P, KD, D])
    t_wo = din("wo8", [HD, HP, 2, KD, P])
    t_w1 = din("w18", [P, KD, F])
    t_w2 = din("w28", [P, FT, D])
    t_bq = din("bq4", [P, KD], f32)
    t_bk = din("bk4", [P, KD], f32)
    t_b1m = din("b1m64", [P, FT], f32)
    t_b1p = din("b1p64", [P, FT], f32)
    t_g1 = din("g1", [P, KD], f32)
    t_be1 = din("be1", [P, KD], f32)
    t_be1p = din("be1p", [P, KD], f32)
    t_g2 = din("g2", [P, KD], f32)
    t_be2 = din("be2", [P, KD], f32)
    t_out = nc.dram_tensor("outT", [P, KD, SQ], bf, kind="ExternalOutput").ap()

    with tile.TileContext(nc) as tc, \
         tc.tile_pool(name="statics", bufs=1) as SP:
        def st(shape, dt, name):
            return SP.tile(shape, dt, tag=name, name=name)

        ones_bf = st([P, 1], bf, "ones_bf")
        nc.gpsimd.memset(ones_bf, 1.0 / D)
        warm_rhs = st([1, SQ], bf, "warm_rhs")
        nc.gpsimd.memset(warm_rhs, 0.0)
        ones_row = st([P, HD], bf, "ones_row")
        nc.gpsimd.memset(ones_row, 1.0)
        ebase = st([P, 2, SQ], bf, "ebase")
        nc.gpsimd.memset(ebase, float(np.exp(EXP_SCALE)))
        nhalf_t = st([1, SQ], f32, "nhalf_t")
        nc.gpsimd.memset(nhalf_t, -0.5)
        eps_w = st([1, 1], bf, "eps_w")
        nc.gpsimd.memset(eps_w, EPS)
        ones_rhs = st([1, SQ], bf, "ones_rhs")
        nc.gpsimd.memset(ones_rhs, 1.0)
        sink0 = st([1, 1], f32, "sink0")
        sink1 = st([1, 1], f32, "sink1")

        # persistent activations
        q_z = st([P, KD, 2, SQ], f8, "q_z")         # slot1 = zeros
        kh = st([P, KD, S + P], f8, "kh")           # +128 zero pad for DR dup
        vh = st([P, KT, H, VC], f8, "vh")
        avt = st([P, H, SQ], f8, "avt")
        xres = st([P, KD, SQ], bf, "xres")          # residual; reused as r2
        x1b = st([P, KD, SQ], f8, "x1b")
        x1f = st([P, KD, SQ], bf, "x1f")
        hsb = st([P, FT, SQ], f8, "hsb")
        outsb = st([P, KD, SQ], bf, "outsb")

        # zero regions via gpsimd (Pool is idle until the first pool-exp)
        nc.gpsimd.memset(q_z[:, :, 1, :], 0.0)
        nc.gpsimd.memset(kh[:, :, S:S + P], 0.0)
        nc.gpsimd.memset(vh[:, :, :, HD + 1:VC], 0.0)

        # ---- DMAs in need-by order; kt/vt interleaved per 512-token block
        bk = st([P, KD], f32, "bk")
        nc.sync.dma_start(out=bk, in_=t_bk)
        bq = st([P, KD], f32, "bq")
        nc.sync.dma_start(out=bq, in_=t_bq)
        wk = st([P, KD, D], f8, "wk")
        nc.sync.dma_start(out=wk, in_=t_wk)
        kt_sb = st([P, KD, S + P], f8, "kt_sb")     # K proj moving data
        vt_sb = st([P, KD, S], f8, "vt_sb")
        nc.sync.dma_start(out=kt_sb[:, :, 0:512], in_=t_kt[:, :, 0:512])
        q8 = st([P, KD, SQ], f8, "q8")
        nc.sync.dma_start(out=q8, in_=t_q8)
        wq = st([P, KD, D], f8, "wq")
        nc.sync.dma_start(out=wq, in_=t_wq)
        wv = st([P, KD, D], f8, "wv")
        nc.sync.dma_start(out=wv, in_=t_wv)
        nc.sync.dma_start(out=vt_sb[:, :, 0:512], in_=t_vt[:, :, 0:512])
        for tb in range(1, TB):
            tbs = slice(tb * 512, (tb + 1) * 512)
            nc.sync.dma_start(out=kt_sb[:, :, tbs], in_=t_kt[:, :, tbs])
            nc.sync.dma_start(out=vt_sb[:, :, tbs], in_=t_vt[:, :, tbs])
        # tail-phase inputs, queued last
        qbf16 = st([P, KD, SQ], bf, "qbf16")
        nc.sync.dma_start(out=qbf16, in_=t_qbf)
        wo = st([HD, HP, 2, KD, P], f8, "wo")
        nc.sync.dma_start(out=wo, in_=t_wo)
        w1 = st([P, KD, F], f8, "w1")
        nc.sync.dma_start(out=w1, in_=t_w1)
        w2 = st([P, FT, D], f8, "w2")
        nc.sync.dma_start(out=w2, in_=t_w2)
        b1m = st([P, FT], f32, "b1m")
        nc.sync.dma_start(out=b1m, in_=t_b1m)
        b1p = st([P, FT], f32, "b1p")
        nc.sync.dma_start(out=b1p, in_=t_b1p)
        g1 = st([P, KD], f32, "g1")
        nc.sync.dma_start(out=g1, in_=t_g1)
        be1 = st([P, KD], f32, "be1")
        nc.sync.dma_start(out=be1, in_=t_be1)
        be1p = st([P, KD], f32, "be1p")
        nc.sync.dma_start(out=be1p, in_=t_be1p)
        g2 = st([P, KD], f32, "g2")
        nc.sync.dma_start(out=g2, in_=t_g2)
        be2 = st([P, KD], f32, "be2")
        nc.sync.dma_start(out=be2, in_=t_be2)
        SP.seal()

        nc.gpsimd.memset(vh[:, :, :, HD:HD + 1], ONES_COL)

        # ============ phases 1+2: projections interleaved with attention
        with tc.tile_pool(name="att_sb", bufs=1) as asb, \
             tc.tile_pool(name="pj", bufs=2, space="PSUM") as pj, \
             tc.tile_pool(name="sc", bufs=1, space="PSUM") as scp, \
             tc.tile_pool(name="av", bufs=1, space="PSUM") as avp:

            def qgroup(dt):
                ps = pj.tile([P, SQ], f32, tag="pj", name=f"psq{dt}")
                for k in (0, 2):
                    nc.tensor.matmul(ps, wq[:, k:k + 2, dt * P:(dt + 1) * P],
                                     q8[:, k:k + 2, :], start=(k == 0),
                                     stop=(k == 2), perf_mode=DR)
                nc.vector.tensor_scalar(out=q_z[:, dt, 0, :], in0=ps,
                                        scalar1=SCALE_QKV, scalar2=bq[:, dt:dt + 1],
                                        op0=ALU.mult, op1=ALU.add)

            def kgroup(dt, tb):
                def go():
                    tbs = slice(tb * 512, (tb + 1) * 512)
                    ps = pj.tile([P, 512], f32, tag="pj", name=f"psk{dt}_{tb}")
                    for k in (0, 2):
                        nc.tensor.matmul(ps, wk[:, k:k + 2, dt * P:(dt + 1) * P],
                                         kt_sb[:, k:k + 2, tbs], start=(k == 0),
                                         stop=(k == 2), perf_mode=DR)
                    nc.vector.tensor_scalar(out=kh[:, dt, tbs], in0=ps,
                                            scalar1=SCALE_QKV,
                                            scalar2=bk[:, dt:dt + 1],
                                            op0=ALU.mult, op1=ALU.add)
                return go

            def qk_groups(dt):
                return [lambda: qgroup(dt)] + [kgroup(dt, tb) for tb in range(TB)]

            def v_proj(tt):
                ps = pj.tile([P, D], f32, tag="pj", name=f"psv{tt}")
                for k in (0, 2):
                    nc.tensor.matmul(ps, vt_sb[:, k:k + 2, tt * P:(tt + 1) * P],
                                     wv[:, k:k + 2, :], start=(k == 0),
                                     stop=(k == 2), perf_mode=DR)
                nc.vector.tensor_scalar_mul(
                    vh[:, tt, :, 0:HD], ps.rearrange("p (h d) -> p h d", h=H),
                    SCALE_QKV)

            # Wo chains: po psum tiles live in the pj pool (projections are
            # done by the time these run).
            po_tiles = {}

            def wo_mm(dt, hp_i):
                def go():
                    if dt not in po_tiles:
                        po_tiles[dt] = pj.tile([P, SQ], f32, tag="pj",
                                               name=f"po{dt}")
                    nc.tensor.matmul(po_tiles[dt], wo[:, hp_i, :, dt, :],
                                     avt[0:HD, 2 * hp_i:2 * hp_i + 2, :],
                                     start=(hp_i == 0), stop=(hp_i == HP - 1),
                                     perf_mode=DR)
                return go

            def wo_consume(dt):
                po = po_tiles.pop(dt)
                nc.vector.scalar_tensor_tensor(
                    out=xres[:, dt, :], in0=po, scalar=SCALE_WO,
                    in1=qbf16[:, dt, :], op0=ALU.mult, op1=ALU.add)

            # keep-warm matmuls to cover the initial DMA wait
            warm_ps = pj.tile([P, SQ], f32, tag="pj", name="warm_ps")
            for w in range(8):
                nc.tensor.matmul(warm_ps[0:1, :], ones_bf[0:1, 0:1], warm_rhs,
                                 start=(w == 0), stop=(w == 7))
            nc.vector.tensor_scalar(out=sink0, in0=warm_ps[0:1, 0:1],
                                    scalar1=0.0, scalar2=0.0,
                                    op0=ALU.mult, op1=ALU.add)

            kgroup(0, 0)()
            qgroup(0)
            for tb in range(1, TB):
                kgroup(0, tb)()
            for g in qk_groups(1):
                g()
            fillers = []

            def normalize(hp_n, pav0_n, pav1_n):
                for side, pav in ((0, pav0_n), (1, pav1_n)):
                    h = 2 * hp_n + side
                    rec = asb.tile([1, SQ], bf, tag="rec", bufs=2,
                                   name=f"rec{h}")
                    with nc.allow_low_precision(reason="softmax denom, bf16 ok"):
                        nc.vector.reciprocal(rec, pav[HD:HD + 1, :])
                    rbc = asb.tile([HD, SQ], bf, tag="rbc", bufs=2,
                                   name=f"rbc{h}")
                    nc.gpsimd.partition_broadcast(rbc, rec)
                    nc.vector.tensor_mul(avt[0:HD, h, :], pav[0:HD, :], rbc)

            prev_pavs = None
            for hp in range(HP):
                if hp == 1:
                    fillers += qk_groups(2)
                elif hp == 2:
                    fillers += qk_groups(3)
                elif hp == 3:
                    fillers += [wo_mm(dt, i) for i in range(3) for dt in (0, 1)]
                pav0 = avp.tile([P, SQ], f32, tag="av0")
                pav1 = avp.tile([P, SQ], f32, tag="av1")
                prev = None
                for k2 in range(K2):
                    pscs = []
                    for side in range(2):
                        psc = scp.tile([P, 2, SQ], f32, tag=f"sc{side}", bufs=1)
                        pr = slice(side * HD, side * HD + HD)
                        for i in range(2):
                            kt = 2 * k2 + i
                            lw = kh[pr, hp, kt * P:(kt + 2) * P].rearrange(
                                "p (two k) -> p two k", two=2)
                            nc.tensor.matmul(psc[:, i, :], lw,
                                             q_z[pr, hp, :, :], start=True,
                                             stop=True, perf_mode=DR)
                        pscs.append(psc)
                    ps_ = []
                    for side in range(2):
                        p = asb.tile([P, 2, SQ], f8, tag=f"p{side}", bufs=2)
                        if k2 in POOL_K2[side]:
                            scb = asb.tile([P, 2, SQ], bf, tag=f"scb{side}",
                                           bufs=2)
                            nc.vector.tensor_copy(scb, pscs[side])
                            nc.gpsimd.tensor_tensor(p, ebase, scb, ALU.pow)
                        else:
                            nc.scalar.activation(out=p, in_=pscs[side],
                                                 func=ACT.Exp, scale=EXP_SCALE)
                        ps_.append(p)
                    if k2 == 0 and prev_pavs is not None:
                        normalize(*prev_pavs)
                        prev_pavs = None
                    if hp == 0:
                        v_proj(2 * k2)
                        v_proj(2 * k2 + 1)
                    elif fillers:
                        fillers.pop(0)()
                    if prev is not None:
                        pp, pk2 = prev
                        for side, pav in ((0, pav0), (1, pav1)):
                            nc.tensor.matmul(
                                pav[0:VC, :],
                                vh[:, 2 * pk2:2 * pk2 + 2, 2 * hp + side, :],
                                pp[side], start=(pk2 == 0),
                                stop=False, perf_mode=DR)
                    prev = (ps_, k2)
                pp, pk2 = prev
                for side, pav in ((0, pav0), (1, pav1)):
                    nc.tensor.matmul(
                        pav[0:VC, :], vh[:, 2 * pk2:2 * pk2 + 2, 2 * hp + side, :],
                        pp[side], start=False, stop=True, perf_mode=DR)
                while hp == 3 and fillers:
                    fillers.pop(0)()
                prev_pavs = (hp, pav0, pav1)

            normalize(*prev_pavs)

            # finish Wo for dt 0/1 (head pair 3) + residual
            for dt in (0, 1):
                wo_mm(dt, 3)()
                wo_consume(dt)

        # ============ phase 3: Wo tail + LN1 ============
        def layer_norm(src, gain, outs, stp, tmp, wmp, nwarm):
            """outs: list of (dst, beta_ap, dtype-handled-by-dst)."""
            ps1 = stp.tile([1, SQ], f32, tag="s1")
            ps2 = stp.tile([1, SQ], f32, tag="s2")
            # seed ps2 with eps so var+eps falls out of the sum chain
            nc.tensor.matmul(ps2, eps_w, ones_rhs, start=True, stop=False)
            for dt in range(KD):
                sq = tmp.tile([P, SQ], bf, tag="sq", bufs=2)
                nc.vector.tensor_mul(sq, src[:, dt, :], src[:, dt, :])
                nc.tensor.matmul(ps1, ones_bf, src[:, dt, :], start=(dt == 0),
                                 stop=(dt == KD - 1))
                nc.tensor.matmul(ps2, ones_bf, sq, start=False,
                                 stop=(dt == KD - 1))
            if nwarm:
                warm = wmp.tile([P, SQ], f32, tag="wm", name="lnwarm")
                for w in range(nwarm):
                    nc.tensor.matmul(warm[0:1, :], ones_bf[0:1, 0:1], warm_rhs,
                                     start=(w == 0), stop=(w == nwarm - 1))
                nc.vector.tensor_scalar(out=sink1, in0=warm[0:1, 0:1],
                                        scalar1=0.0, scalar2=0.0,
                                        op0=ALU.mult, op1=ALU.add)
            mean = tmp.tile([1, SQ], f32, tag="ln_mean")
            nc.vector.tensor_copy(mean, ps1)
            m2 = tmp.tile([1, SQ], f32, tag="ln_m2")
            nc.vector.tensor_mul(m2, mean, mean)
            var = tmp.tile([1, SQ], f32, tag="ln_var")
            nc.vector.scalar_tensor_tensor(out=var, in0=m2, scalar=-1.0,
                                           in1=ps2, op0=ALU.mult, op1=ALU.add)
            rstd = tmp.tile([1, SQ], bf, tag="ln_rstd")
            with nc.allow_low_precision(reason="LN rstd, bf16 ok"):
                nc.gpsimd.tensor_tensor(rstd, var, nhalf_t, ALU.pow)
            cvec = tmp.tile([1, SQ], bf, tag="ln_c")
            nc.vector.tensor_mul(cvec, mean, rstd)
            pA = tmp.tile([P, SQ], bf, tag="bA")
            nc.gpsimd.partition_broadcast(pA, rstd)
            pC = tmp.tile([P, SQ], bf, tag="bC")
            nc.gpsimd.partition_broadcast(pC, cvec)
            for dt in range(KD):
                t1 = tmp.tile([P, SQ], bf, tag="t1", bufs=2)
                nc.vector.tensor_mul(t1, src[:, dt, :], pA)
                nc.vector.tensor_sub(t1, t1, pC)
                for dst, beta in outs:
                    nc.vector.tensor_scalar(out=dst[:, dt, :], in0=t1,
                                            scalar1=gain[:, dt:dt + 1],
                                            scalar2=beta[:, dt:dt + 1],
                                            op0=ALU.mult, op1=ALU.add)

        with tc.tile_pool(name="ln1_sb", bufs=1) as tmp1, \
             tc.tile_pool(name="po2", bufs=2, space="PSUM") as pop, \
             tc.tile_pool(name="st1", bufs=1, space="PSUM") as stp1, \
             tc.tile_pool(name="wm1", bufs=1, space="PSUM") as wmp1:
            for dt in (2, 3):
                po_tiles[dt] = pop.tile([P, SQ], f32, tag="po", name=f"po{dt}")
                for hp_i in range(HP):
                    nc.tensor.matmul(po_tiles[dt], wo[:, hp_i, :, dt, :],
                                     avt[0:HD, 2 * hp_i:2 * hp_i + 2, :],
                                     start=(hp_i == 0), stop=(hp_i == HP - 1),
                                     perf_mode=DR)
                wo_consume(dt)
            layer_norm(xres, g1, [(x1b, be1), (x1f, be1p)], stp1, tmp1, wmp1, 14)

        # ============ phase 4: FFN ============
        with tc.tile_pool(name="pf", bufs=5, space="PSUM") as pfp, \
             tc.tile_pool(name="py", bufs=2, space="PSUM") as pyp, \
             tc.tile_pool(name="wm2", bufs=1, space="PSUM") as wmp2:
            for ft in range(FT):
                pf = pfp.tile([P, SQ], f32, tag="pf")
                for k in (0, 2):
                    nc.tensor.matmul(pf, w1[:, k:k + 2, ft * P:(ft + 1) * P],
                                     x1b[:, k:k + 2, :], start=(k == 0),
                                     stop=(k == 2), perf_mode=DR)
                eng = FT_ENG[ft]
                if eng == "a":
                    nc.scalar.activation(out=hsb[:, ft, :], in_=pf, func=ACT.Relu,
                                         bias=b1p[:, ft:ft + 1])
                else:
                    e = nc.vector if eng == "d" else nc.gpsimd
                    e.tensor_scalar(out=hsb[:, ft, :], in0=pf,
                                    scalar1=b1m[:, ft:ft + 1], scalar2=SCALE_QKV,
                                    op0=ALU.max, op1=ALU.mult)
            for dt in range(KD):
                py = pyp.tile([P, SQ], f32, tag="py")
                for f in range(0, FT, 2):
                    nc.tensor.matmul(py, w2[:, f:f + 2, dt * P:(dt + 1) * P],
                                     hsb[:, f:f + 2, :], start=(f == 0),
                                     stop=(f == FT - 2), perf_mode=DR)
                nc.vector.scalar_tensor_tensor(
                    out=xres[:, dt, :], in0=py, scalar=SCALE_FF2,
                    in1=x1f[:, dt, :], op0=ALU.mult, op1=ALU.add)

        # ============ phase 5: LN2 + output ============
        with tc.tile_pool(name="ln2_sb", bufs=1) as tmp2, \
             tc.tile_pool(name="st2", bufs=1, space="PSUM") as stp2, \
             tc.tile_pool(name="wm3", bufs=1, space="PSUM") as wmp3:
            layer_norm(xres, g2, [(outsb, be2)], stp2, tmp2, wmp3, 0)
            for dt in range(KD):
                nc.sync.dma_start(out=t_out[:, dt, :], in_=outsb[:, dt, :])

    nc.compile()
    return nc


def _get_nc():
    if "nc" not in _CACHE:
        _CACHE["nc"] = _build_nc()
    return _CACHE["nc"]


def make_in_maps(q, k, v, Wq, bq, Wk, bk, Wv, bv, Wo, bo, W1, b1, W2, b2,
                 g1, be1, g2, be2):
    f32 = np.float32
    q = np.asarray(q, f32)
    k = np.asarray(k, f32)
    v = np.asarray(v, f32)
    Wq, Wk, Wv, Wo = (np.asarray(x, f32) for x in (Wq, Wk, Wv, Wo))
    W1, W2 = np.asarray(W1, f32), np.asarray(W2, f32)
    bq, bk, bv, bo = (np.asarray(x, f32) for x in (bq, bk, bv, bo))
    b1, b2 = np.asarray(b1, f32), np.asarray(b2, f32)
    g1, be1, g2, be2 = (np.asarray(x, f32) for x in (g1, be1, g2, be2))

    def tile_pd(x, n):  # [n*P] -> [P, n]
        return np.ascontiguousarray(x.reshape(n, P).T)

    def wt8(w, cols):  # [out, in] -> [P, in//P, cols] scaled fp8
        return np.ascontiguousarray(
            (w.T * WS).reshape(-1, P, cols).transpose(1, 0, 2)).astype(F8)

    # per-row (hidden-unit) scale for W2: ACT-relu rows hold 64h -> x4,
    # max-trick rows hold 4t -> x64
    row_scale = np.empty(F, f32)
    t_mask = np.zeros(F, bool)
    for ft in range(FT):
        sl = slice(ft * P, (ft + 1) * P)
        if FT_ENG[ft] == "a":
            row_scale[sl] = 4.0
        else:
            row_scale[sl] = 64.0
            t_mask[sl] = True
    w28 = np.ascontiguousarray(
        (W2.T * row_scale[:, None]).reshape(FT, P, D).transpose(1, 0, 2)).astype(F8)

    bo_eff = bo + Wo @ bv
    b2_eff = b2 + W2[:, t_mask] @ b1[t_mask]

    wo8 = np.ascontiguousarray(
        (Wo.T * WS).reshape(HP, 2, HD, KD, P).transpose(2, 0, 1, 3, 4)).astype(F8)

    shared = {
        "wq8": wt8(Wq, D), "wk8": wt8(Wk, D), "wv8": wt8(Wv, D),
        "w18": wt8(W1, F), "w28": w28, "wo8": wo8,
        "bq4": tile_pd(4.0 * bq, KD), "bk4": tile_pd(4.0 * bk, KD),
        "b1m64": tile_pd(-64.0 * b1, FT), "b1p64": tile_pd(64.0 * b1, FT),
        "g1": tile_pd(g1, KD), "be1": tile_pd(be1, KD),
        "be1p": tile_pd(be1 + b2_eff, KD),
        "g2": tile_pd(g2, KD), "be2": tile_pd(be2, KD),
    }

    def fm(x, dt):  # [S, D] -> [P, KD, S] feature-major
        return np.ascontiguousarray(
            x.T.reshape(KD, P, -1).transpose(1, 0, 2)).astype(dt)

    kts = [fm(k[b], F8) for b in range(B)]
    vts = [fm(v[b], F8) for b in range(B)]

    in_maps = []
    for c in range(NCORES):
        b, s0 = c // 4, (c % 4) * SQ
        qs = q[b, s0:s0 + SQ, :]
        in_maps.append({
            "q8": fm(qs, F8),
            "qbf16": fm(qs + bo_eff, BF16),
            "kt8": kts[b], "vt8": vts[b], **shared,
        })
    return in_maps


def assemble_out(results):
    out = np.empty((B, S, D), np.float32)
    for c in range(NCORES):
        b, s0 = c // 4, (c % 4) * SQ
        out[b, s0:s0 + SQ, :] = results[c]["outT"].astype(np.float32) \
            .transpose(2, 1, 0).reshape(SQ, D)
    return out


def kernel(**inputs):
    global LAST_RESULT
    import os

    from concourse.bass_utils import run_bass_kernel_spmd

    nc = _get_nc()
    in_maps = make_in_maps(**inputs)
    try:
        res = run_bass_kernel_spmd(nc, in_maps, core_ids=list(range(NCORES)))
    except ModuleNotFoundError:
        # BASS_TRACE set but this container has no axon NTFF profile hook
        # (antenv.axon_hooks missing) — rerun untraced.
        os.environ["BASS_NEVER_TRACE"] = "1"
        res = run_bass_kernel_spmd(nc, in_maps, core_ids=list(range(NCORES)))
    LAST_RESULT = res
    return assemble_out(res.results)
